# revision 1
# baseline (speedup 1.0000x reference)
"""Trainium2 Bass kernel for the neural-renderer silhouette MSE loss.

Reference computation: project 512 vertices, gather 1024 triangle faces,
rasterize a 256x256 silhouette (a pixel is covered iff it lies strictly
inside some valid face and the perspective-correct depth is in (NEAR, FAR)),
then return sum((sil - image_ref)^2).

Reformulation: each barycentric weight w_i of face f is an *affine* function
of the pixel NDC coords, w_i = a_i*x + b_i*y + c_i, so
    covered(p) = max_f min_i w_i(p, f) > 0.
The depth test is provably redundant when every camera-space vertex z lies
inside (NEAR, FAR); otherwise two extra affine maps are appended to the min.

Work pruning (host-side, exact):
  - A pixel strictly outside the global face bounding box can never be
    covered; its loss term ref^2 is summed on the host.
  - The in-bbox area is cut into 16x8-pixel blocks (= one 128-lane tile
    each). Each block only needs faces whose bbox overlaps it (~20 median,
    vs 1024). Blocks are sorted by face count and snake-dealt to the 8
    cores, so all cores run an identical slot schedule (SPMD) whose per-slot
    face capacity is the max count in the 8-block group.

Device (SPMD, one program on 8 cores; schedule baked at build time):
  - PE: per (slot, chunk): one K=9 bf16 matmul per affine map
        (lhsT = pixel matrix [9, 128], rhs = coefficients [9, ch]) -> PSUM.
    Each fp32 coefficient is split into 3 bf16 components (exact to ~2^-25);
    pixel coords (2i+1-256)/256 are exactly representable in bf16, so fp32
    PSUM accumulation reproduces fp32 affine values essentially exactly.
  - ACT: stages map 0 PSUM->SBUF as bf16 (sign-exact suffices) because the
    DVE reads at most one PSUM operand per instruction.
  - DVE: tensor_tensor mins + reduce_max over faces, then an epilogue
    computing sum((cov>0) - ref)^2 per partition row.
  - Host: sums 8x128 partials + the out-of-bbox ref^2 term.
"""

import os
import sys
from contextlib import ExitStack

import numpy as np

for _p in (
    "/opt/trn_rl_repo",
    "/root/.axon_site",
    "/root/.axon_site/_ro/trn_rl_repo",
    "/root/.axon_site/_ro/pypackages",
):
    if os.path.isdir(_p) and _p not in sys.path:
        sys.path.append(_p)

import ml_dtypes  # noqa: E402

import concourse.bacc as bacc  # noqa: E402
import concourse.bass as bass  # noqa: E402
import concourse.tile as tile  # noqa: E402
from concourse import mybir  # noqa: E402
from concourse.alu_op_type import AluOpType  # noqa: E402
from concourse.bass_utils import run_bass_kernel_spmd  # noqa: E402

IS = 256
NEAR, FAR = 0.1, 100.0
VIEW_ANGLE_DEG = 30.0
CAM_DIST, ELEV, AZIM = 2.732, 0.0, 90.0
EPS = 1e-9

NCORES = 8
PTILE = 128                  # pixels per tile slot (partition dim)
BH, BW = 16, 8               # pixel block shape (BH*BW == PTILE)
MAXCHUNK = 512               # max matmul free size / PSUM bank
KSPLIT = 3                   # bf16 components per fp32 coefficient
K = 3 * KSPLIT               # matmul contraction dim
DUMMY_XY = -4.0              # off-screen coord for padding pixels

_prog_cache: dict = {}


class LeanTileContext(tile.TileContext):
    """TileContext with a cheaper end-of-kernel sequence.

    The stock _drain_and_barrier emits drain + full all-engine barrier +
    semaphore clear + a second full barrier (~10us measured). The drain
    already waits for every engine/DMA via the global clock; a sem-only
    barrier suffices to order the semaphore clear, and the trailing barrier
    only guards re-execution races that the NEFF-end quiesce covers anyway.
    """

    def _drain_and_barrier(self, tick_clock, wait_clock):
        from concourse.tile import ScopedClock

        drain_inst = self.nc.sync.drain()
        wait_clock.add_sem_waits(
            drain_inst.ins, ScopedClock({None: tick_clock.global_clock}))
        self.nc.all_engine_barrier(sem_only=True)
        popped = self.nc._tile_sem_poison_stack.pop()
        assert popped is self._sem_poison
        self.nc.clear_and_free_semaphores(
            list(self.sems.allocated().values()))
        self.nc.all_engine_barrier(sem_only=True)


def _camera_transform(v: np.ndarray) -> np.ndarray:
    """Replicate reference's look_at + perspective in fp32. v: [V,3]."""
    e, a = np.radians(ELEV), np.radians(AZIM)
    eye = np.array(
        [
            CAM_DIST * np.cos(e) * np.sin(a),
            CAM_DIST * np.sin(e),
            -CAM_DIST * np.cos(e) * np.cos(a),
        ],
        dtype=np.float32,
    )
    at = np.zeros(3, np.float32)
    up = np.array([0.0, 1.0, 0.0], np.float32)
    z = at - eye
    z = (z / np.linalg.norm(z)).astype(np.float32)
    x = np.cross(up, z)
    x = (x / np.linalg.norm(x)).astype(np.float32)
    y = np.cross(z, x)
    y = (y / np.linalg.norm(y)).astype(np.float32)
    R = np.stack([x, y, z]).astype(np.float32)
    vc = ((v - eye) @ R.T).astype(np.float32)
    w = np.float32(np.tan(np.radians(VIEW_ANGLE_DEG)))
    zc = vc[:, 2]
    return np.stack([vc[:, 0] / (zc * w), vc[:, 1] / (zc * w), zc], -1).astype(
        np.float32
    )


def _face_coefficients(fv: np.ndarray):
    """Affine coefficients per map: returns (coeffs [nmaps,3,F] f32,
    valid [F] bool, nmaps)."""
    F = fv.shape[0]
    x0, x1, x2 = fv[:, 0, 0], fv[:, 1, 0], fv[:, 2, 0]
    y0, y1, y2 = fv[:, 0, 1], fv[:, 1, 1], fv[:, 2, 1]
    z0, z1, z2 = fv[:, 0, 2], fv[:, 1, 2], fv[:, 2, 2]

    denom = (y1 - y2) * (x0 - x2) + (x2 - x1) * (y0 - y2)
    valid = (np.abs(denom) > EPS) & np.all(np.isfinite(fv.reshape(F, -1)), -1)
    d = np.where(valid, denom, np.float32(1.0)).astype(np.float32)

    a0 = (y1 - y2) / d
    b0 = (x2 - x1) / d
    c0 = -(a0 * x2 + b0 * y2)
    a1 = (y2 - y0) / d
    b1 = (x0 - x2) / d
    c1 = -(a1 * x2 + b1 * y2)
    a2 = -(a0 + a1)
    b2 = -(b0 + b1)
    c2 = np.float32(1.0) - c0 - c1

    # Depth redundancy: for an interior pixel the perspective-correct depth
    # is a harmonic mean of vertex z's, hence inside (NEAR, FAR) whenever
    # all (valid-face) vertex z's are.
    z_valid = fv[valid][:, :, 2] if valid.any() else np.array([[1.0]])
    depth_safe = bool(
        np.all((z_valid > NEAR * 1.0001) & (z_valid < FAR * 0.9999)))

    maps = [(a0, b0, c0), (a1, b1, c1), (a2, b2, c2)]
    if not depth_safe:
        iz0 = np.float32(1.0) / z0
        iz1 = np.float32(1.0) / z1
        iz2 = np.float32(1.0) / z2
        az = a0 * iz0 + a1 * iz1 + a2 * iz2
        bz = b0 * iz0 + b1 * iz1 + b2 * iz2
        cz = c0 * iz0 + c1 * iz1 + c2 * iz2
        maps.append((az, bz, cz - np.float32(1.0 / FAR)))
        maps.append((-az, -bz, np.float32(1.0 / NEAR) - cz))

    nmaps = len(maps)
    coeffs = np.empty((nmaps, 3, F), np.float32)
    for m, (a, b, c) in enumerate(maps):
        bad = ~(valid & np.isfinite(a) & np.isfinite(b) & np.isfinite(c))
        coeffs[m, 0] = np.where(bad, np.float32(0.0), a)
        coeffs[m, 1] = np.where(bad, np.float32(0.0), b)
        coeffs[m, 2] = np.where(bad, np.float32(-1.0), c)
    return coeffs, valid, nmaps


def _split_bf16(v: np.ndarray) -> list[np.ndarray]:
    """Split fp32 array into KSPLIT bf16 components summing to ~v (2^-25)."""
    parts = []
    rem = v.astype(np.float32)
    for _ in range(KSPLIT):
        p = rem.astype(ml_dtypes.bfloat16)
        parts.append(p)
        rem = (rem - p.astype(np.float32)).astype(np.float32)
    return parts


def _make_schedule(vertices, image_ref, faces):
    """Host planning: prune + block + deal. Returns (in_maps, nmaps,
    chunks_per_slot, host_extra)."""
    v = np.asarray(vertices, np.float32)[0]
    f = np.asarray(faces)[0].astype(np.int64)
    img = np.asarray(image_ref, np.float32)[0]
    img_flat = img.reshape(-1)

    vp = _camera_transform(v)
    fv = vp[f]                                    # [F,3,3]
    coeffs, valid, nmaps = _face_coefficients(fv)
    F = fv.shape[0]

    i = np.arange(IS, dtype=np.float32)
    xcol = (2.0 * i + 1.0 - IS) / IS
    yrow = (2.0 * (IS - 1.0 - i) + 1.0 - IS) / IS   # decreasing in row
    marg = np.float32(2.0 / IS)                     # one-pixel margin

    vi = np.where(valid)[0]
    if len(vi):
        fx = fv[:, :, 0]
        fy = fv[:, :, 1]
        fxmin, fxmax = fx.min(1), fx.max(1)
        fymin, fymax = fy.min(1), fy.max(1)
        gxmin, gxmax = fxmin[vi].min(), fxmax[vi].max()
        gymin, gymax = fymin[vi].min(), fymax[vi].max()
        rows = np.where((yrow >= gymin - marg) & (yrow <= gymax + marg))[0]
        cols = np.where((xcol >= gxmin - marg) & (xcol <= gxmax + marg))[0]
    else:
        rows = cols = np.array([], np.int64)

    blocks = []   # (count, face_idx_array, pixel_idx_array (len<=128))
    if len(rows) and len(cols):
        r0, r1 = int(rows.min()), int(rows.max()) + 1
        c0, c1 = int(cols.min()), int(cols.max()) + 1
        for rr in range(r0, r1, BH):
            for cc in range(c0, c1, BW):
                rr2, cc2 = min(rr + BH, r1), min(cc + BW, c1)
                ylo, yhi = yrow[rr2 - 1] - marg, yrow[rr] + marg
                xlo, xhi = xcol[cc] - marg, xcol[cc2 - 1] + marg
                inter = valid & (fymax >= ylo) & (fymin <= yhi) \
                    & (fxmax >= xlo) & (fxmin <= xhi)
                fl = np.where(inter)[0]
                rgrid, cgrid = np.meshgrid(np.arange(rr, rr2),
                                           np.arange(cc, cc2), indexing="ij")
                px = (rgrid * IS + cgrid).reshape(-1)
                blocks.append((len(fl), fl, px))

    if not blocks:
        blocks = [(0, np.array([], np.int64), np.array([], np.int64))]

    blocks.sort(key=lambda b: -b[0])
    NT = (len(blocks) + NCORES - 1) // NCORES
    empty = (0, np.array([], np.int64), np.array([], np.int64))
    while len(blocks) < NT * NCORES:
        blocks.append(empty)

    # slot capacities and chunk splits (shared across cores)
    chunks_per_slot = []
    for j in range(NT):
        grp = blocks[NCORES * j:NCORES * (j + 1)]
        cap = max(32, int(np.ceil(max(b[0] for b in grp) / 32)) * 32)
        nch = (cap + MAXCHUNK - 1) // MAXCHUNK
        ch = int(np.ceil(cap / nch / 32)) * 32
        chunks_per_slot.append((ch,) * nch)
    # descending slot order keeps the PE-bound packs at the schedule tail
    # (overlapping the big slots' longer DVE chains)
    order = list(range(NT))
    chunks_per_slot = tuple(chunks_per_slot[g] for g in order)
    CTOT = sum(sum(c) for c in chunks_per_slot)

    # coefficient splits with a trailing dummy column (index F)
    csp = np.empty((nmaps, 3, KSPLIT, F + 1), ml_dtypes.bfloat16)
    for m in range(nmaps):
        for j3 in range(3):
            col = np.concatenate(
                [coeffs[m, j3],
                 [np.float32(-1.0 if j3 == 2 else 0.0)]])
            for s, part in enumerate(_split_bf16(col)):
                csp[m, j3, s] = part

    assigned = np.zeros(IS * IS, bool)
    in_maps = []
    for k in range(NCORES):
        pix = np.full((K, NT * PTILE), np.float32(DUMMY_XY), np.float32)
        ref = np.zeros((PTILE, NT), np.float32)
        coef = np.empty((K, nmaps * CTOT), ml_dtypes.bfloat16)
        colbase = 0
        for j in range(NT):
            cnt, fl, px = blocks[NCORES * order[j] + k]
            # pixels
            npx = len(px)
            if npx:
                lane_x = xcol[px % IS]
                lane_y = yrow[px // IS]
                for s in range(KSPLIT):
                    pix[s * 3 + 0, j * PTILE:j * PTILE + npx] = lane_x
                    pix[s * 3 + 1, j * PTILE:j * PTILE + npx] = lane_y
                ref[:npx, j] = img_flat[px]
                assigned[px] = True
            for s in range(KSPLIT):
                pix[s * 3 + 2, j * PTILE:(j + 1) * PTILE] = 1.0
            # faces (padded with dummy index F)
            capj = sum(chunks_per_slot[j])
            fidx = np.full(capj, F, np.int64)
            fidx[:cnt] = fl
            pos = 0
            for ch in chunks_per_slot[j]:
                sel = fidx[pos:pos + ch]
                for m in range(nmaps):
                    for s in range(KSPLIT):
                        for j3 in range(3):
                            coef[s * 3 + j3,
                                 colbase + m * ch:colbase + (m + 1) * ch] = \
                                csp[m, j3, s][sel]
                colbase += nmaps * ch
                pos += ch
        in_maps.append({
            "coef": np.concatenate(
                [pix.astype(ml_dtypes.bfloat16), coef], axis=1),
            "ref": ref,
        })

    host_extra = float(np.sum(np.square(img_flat[~assigned]),
                              dtype=np.float32))
    return in_maps, nmaps, chunks_per_slot, host_extra


def _work_items(nmaps: int, chunks_per_slot):
    """Group slots into device work items.

    ("p", cap, S, j0): S consecutive equal-cap single-chunk slots whose
    nmaps*cap*S columns fit one PSUM bank group -> merged matmuls + one
    strided DVE min/reduce pass for all S slots.
    ("s", j): one slot processed chunk-by-chunk with per-map matmuls.
    Packing is disabled for nmaps=5 (PSUM budget).
    """
    items = []
    NT = len(chunks_per_slot)
    j = 0
    while j < NT:
        chs = chunks_per_slot[j]
        cap = chs[0]
        if nmaps == 3 and len(chs) == 1 and nmaps * cap <= MAXCHUNK:
            run = 1
            while (j + run < NT and chunks_per_slot[j + run] == chs
                   and nmaps * cap * (run + 1) <= MAXCHUNK):
                run += 1
            items.append(("p", cap, run, j))
            j += run
            continue
        items.append(("s", j))
        j += 1
    return items


def _build_program(nmaps: int, chunks_per_slot) -> bass.Bass:
    NT = len(chunks_per_slot)
    CTOT = sum(sum(c) for c in chunks_per_slot)
    nc = bacc.Bacc()
    PIXW = NT * PTILE
    coef_d = nc.dram_tensor("coef", [K, PIXW + nmaps * CTOT],
                            mybir.dt.bfloat16, kind="ExternalInput")
    ref_d = nc.dram_tensor("ref", [PTILE, NT], mybir.dt.float32,
                           kind="ExternalInput")
    out_d = nc.dram_tensor("out", [PTILE, 1], mybir.dt.float32,
                           kind="ExternalOutput")

    # slot -> coef column span (in the nmaps*CTOT axis)
    slot_cols = []
    cb = 0
    for j in range(NT):
        w = nmaps * sum(chunks_per_slot[j])
        slot_cols.append((cb, cb + w))
        cb += w

    # group slots into DMA parts with progressively larger widths: the first
    # part is small so the earliest slots' matmuls start ASAP while the rest
    # of the coefficients stream in on parallel queues
    NPART = min(4, NT)
    fracs = [0.0, 0.25, 0.5, 0.75, 1.0][:NPART] + [1.0]
    bounds = [cb * f for f in fracs]
    part_of_slot = []
    for j in range(NT):
        g = 0
        while g + 1 < NPART and slot_cols[j][0] >= bounds[g + 1]:
            g += 1
        part_of_slot.append(g)
    part_ranges = []
    for g in range(NPART):
        sl = [j for j in range(NT) if part_of_slot[j] == g]
        if sl:
            part_ranges.append((slot_cols[sl[0]][0], slot_cols[sl[-1]][1]))
        else:
            part_ranges.append(None)

    with LeanTileContext(nc) as tc:
        with ExitStack() as ctx:
            const = ctx.enter_context(tc.tile_pool(name="const", bufs=1))
            # part0 carries the pixel matrix plus the earliest coef columns
            # in a single transfer; remaining parts stream on parallel queues
            issue_engines = [nc.sync, nc.scalar]
            coef_parts = []
            pix_s = None
            for g, rng in enumerate(part_ranges):
                if rng is None:
                    coef_parts.append(None)
                    continue
                lo, hi = rng
                clo = 0 if g == 0 else PIXW + lo
                chi = PIXW + hi
                cp = const.tile([K, chi - clo], mybir.dt.bfloat16,
                                name=f"coefp{g}")
                issue_engines[g % len(issue_engines)].dma_start(
                    cp[:], coef_d[:, clo:chi])
                coef_parts.append((cp, lo))
                if g == 0:
                    pix_s = cp[:, 0:PIXW]
            ref_s = const.tile([PTILE, NT], mybir.dt.float32)
            nc.scalar.dma_start(ref_s[:], ref_d[:])
            mx = const.tile([PTILE, NT], mybir.dt.float32)
            nextra = sum(len(c) - 1 for c in chunks_per_slot)
            extra = const.tile([PTILE, max(nextra, 1)], mybir.dt.float32)

            psum = ctx.enter_context(
                tc.tile_pool(name="psum", bufs=2, space="PSUM"))
            tmp = ctx.enter_context(tc.tile_pool(name="tmp", bufs=3))

            eidx = 0
            for item in _work_items(nmaps, chunks_per_slot):
                if item[0] == "p":
                    _, cap, S, j0 = item
                    wp = psum.tile([PTILE, MAXCHUNK], mybir.dt.float32,
                                   tag="w0", bufs=2)
                    for s in range(S):
                        j = j0 + s
                        lhsT = pix_s[:, j * PTILE:(j + 1) * PTILE]
                        g = part_of_slot[j]
                        cpart, cplo = coef_parts[g]
                        lo = slot_cols[j][0] - cplo + (PIXW if g == 0 else 0)
                        nc.tensor.matmul(
                            wp[:, s * nmaps * cap:(s + 1) * nmaps * cap],
                            lhsT, cpart[:, lo:lo + nmaps * cap],
                            start=True, stop=True)
                    # [128, S, nmaps*cap] view; per-map slice on last axis
                    wv = wp[:, :S * nmaps * cap].rearrange(
                        "p (s mb) -> p s mb", mb=nmaps * cap)
                    w0c = tmp.tile([PTILE, MAXCHUNK], mybir.dt.bfloat16,
                                   tag="w0c")
                    w0v = w0c[:, :S * cap].rearrange("p (s b) -> p s b",
                                                     b=cap)
                    nc.scalar.copy(w0v, wv[:, :, 0:cap])
                    mn = tmp.tile([PTILE, MAXCHUNK], mybir.dt.bfloat16,
                                  tag="mn")
                    mnv = mn[:, :S * cap].rearrange("p (s b) -> p s b", b=cap)
                    nc.vector.tensor_tensor(mnv, w0v, wv[:, :, cap:2 * cap],
                                            op=AluOpType.min)
                    for m in range(2, nmaps):
                        nc.vector.tensor_tensor(
                            mnv, mnv, wv[:, :, m * cap:(m + 1) * cap],
                            op=AluOpType.min)
                    nc.vector.reduce_max(mx[:, j0:j0 + S], mnv,
                                         axis=mybir.AxisListType.X)
                    continue
                j = item[1]
                lhsT = pix_s[:, j * PTILE:(j + 1) * PTILE]
                g = part_of_slot[j]
                cpart, cplo = coef_parts[g]
                cplo -= PIXW if g == 0 else 0
                colbase = slot_cols[j][0]
                for ci, ch in enumerate(chunks_per_slot[j]):
                    ws = []
                    for m in range(nmaps):
                        w = psum.tile([PTILE, MAXCHUNK], mybir.dt.float32,
                                      tag=f"w{m}", bufs=(2 if m < 3 else 1))
                        lo = colbase - cplo + m * ch
                        rhs = cpart[:, lo:lo + ch]
                        nc.tensor.matmul(w[:, :ch], lhsT, rhs,
                                         start=True, stop=True)
                        ws.append(w)
                    colbase += nmaps * ch
                    # ACT stages map0 (DVE: single PSUM operand per inst)
                    w0c = tmp.tile([PTILE, MAXCHUNK], mybir.dt.bfloat16,
                                   tag="w0c")
                    nc.scalar.copy(w0c[:, :ch], ws[0][:, :ch])
                    mn = tmp.tile([PTILE, MAXCHUNK], mybir.dt.bfloat16,
                                  tag="mn")
                    nc.vector.tensor_tensor(mn[:, :ch], w0c[:, :ch],
                                            ws[1][:, :ch], op=AluOpType.min)
                    for m in range(2, nmaps):
                        nc.vector.tensor_tensor(mn[:, :ch], mn[:, :ch],
                                                ws[m][:, :ch],
                                                op=AluOpType.min)
                    if ci == 0:
                        dst = mx[:, j:j + 1]
                    else:
                        dst = extra[:, eidx:eidx + 1]
                    nc.vector.reduce_max(dst, mn[:, :ch],
                                         axis=mybir.AxisListType.X)
                    if ci > 0:
                        nc.vector.tensor_tensor(mx[:, j:j + 1], mx[:, j:j + 1],
                                                extra[:, eidx:eidx + 1],
                                                op=AluOpType.max)
                        eidx += 1

            # diff = (mx > 0 ? 1.0 : 0.0) - ref ; out = rowsum(diff^2)
            diff = const.tile([PTILE, NT], mybir.dt.float32)
            nc.vector.scalar_tensor_tensor(
                out=diff[:], in0=mx[:], scalar=0.0, in1=ref_s[:],
                op0=AluOpType.is_gt, op1=AluOpType.subtract)
            sq = const.tile([PTILE, NT], mybir.dt.float32)
            nc.vector.tensor_tensor(sq[:], diff[:], diff[:],
                                    op=AluOpType.mult)
            losscol = const.tile([PTILE, 1], mybir.dt.float32)
            nc.vector.reduce_sum(losscol[:], sq[:],
                                 axis=mybir.AxisListType.X)
            nc.scalar.dma_start(out_d[:], losscol[:])
    nc.compile()
    return nc


def run_sharded(vertices, image_ref, faces, trace=False, **spmd_kwargs):
    """Runs the SPMD kernel on 8 cores; returns (loss, BassKernelResults)."""
    in_maps, nmaps, chunks, host_extra = _make_schedule(
        vertices, image_ref, faces)
    key = (nmaps, chunks)
    if key not in _prog_cache:
        _prog_cache[key] = _build_program(nmaps, chunks)
    nc = _prog_cache[key]
    results = run_bass_kernel_spmd(
        nc, in_maps, core_ids=list(range(NCORES)), trace=trace, **spmd_kwargs)
    partials = np.stack([r["out"].reshape(-1) for r in results.results])
    loss = np.float32(partials.astype(np.float32).sum(dtype=np.float32)
                      + np.float32(host_extra))
    return loss, results


def kernel(vertices: np.ndarray, image_ref: np.ndarray,
           faces: np.ndarray) -> np.ndarray:
    loss, _ = run_sharded(vertices, image_ref, faces, trace=False)
    return np.asarray(loss, dtype=np.float32)



# revision 6
# speedup vs baseline: 1.7934x; 1.7934x over previous
"""Trainium2 Bass kernel for the neural-renderer silhouette MSE loss.

Reference computation: project 512 vertices, gather 1024 triangle faces,
rasterize a 256x256 silhouette (a pixel is covered iff it lies strictly
inside some valid face and the perspective-correct depth is in (NEAR, FAR)),
then return sum((sil - image_ref)^2).

Reformulation: each barycentric weight w_i of face f is an *affine* function
of the pixel NDC coords, w_i = a_i*x + b_i*y + c_i, so
    covered(p) = max_f min_i w_i(p, f) > 0.
The depth test is provably redundant when every camera-space vertex z lies
inside (NEAR, FAR); otherwise two extra affine maps are appended to the min.

Work pruning (host-side, exact):
  - A pixel strictly outside the global face bounding box can never be
    covered; its loss term ref^2 is summed on the host.
  - The in-bbox area is cut into 16x8-pixel blocks (= one 128-lane tile
    each). Each block only needs faces whose bbox overlaps it.
  - A block that lies fully inside a single valid face (all affine maps
    strictly positive at the block's 4 corner pixel centers, hence by
    linearity at every interior pixel center) is fully covered: the host
    adds sum((1-ref)^2) for it and the device never sees it.  This culls
    the (large) silhouette interior, leaving only boundary-ish blocks.
  - Remaining active blocks are sorted by face count and snake-dealt to
    the 8 cores, so all cores run an identical slot schedule (SPMD) whose
    per-slot face capacity is the max count in the 8-block group (pad 8).

Device (SPMD, one program on 8 cores; schedule baked at build time):
  - PE: per slot one K=9 bf16 matmul (lhsT = pixel matrix [9, 128],
    rhs = coefficients [9, 3*cap]) -> PSUM.  Small slots are packed S per
    PSUM bank (3*cap*S <= 504); large slots use a 3-bank tile with one
    matmul per affine map.  Each fp32 coefficient is split into 3 bf16
    components (exact to ~2^-25); pixel coords are exactly representable
    in bf16, so fp32 PSUM accumulation reproduces fp32 affine values
    essentially exactly.
  - ACT: stages maps 0,1 PSUM->SBUF as bf16 in one strided copy (the DVE
    reads at most one PSUM operand per instruction; sign-exact suffices).
  - DVE: min(w0,w1) at 2x (both SBUF bf16), min(.,w2) vs PSUM, grouped
    reduce_max over faces -> per-slot maxima; epilogue computes
    sum(((mx>0) - ref)^2) per partition row.
  - No end-of-kernel drain/barrier/sem-clear: the NEFF-end quiesce covers
    DMA completion, and the runtime reinitializes semaphores per load
    (verified empirically over repeated executions).
  - Host: sums 8x128 partials + the culled blocks' closed-form terms.
"""

import os
import sys
from contextlib import ExitStack

import numpy as np

for _p in (
    "/opt/trn_rl_repo",
    "/root/.axon_site",
    "/root/.axon_site/_ro/trn_rl_repo",
    "/root/.axon_site/_ro/pypackages",
):
    if os.path.isdir(_p) and _p not in sys.path:
        sys.path.append(_p)

import ml_dtypes  # noqa: E402

import concourse.bacc as bacc  # noqa: E402
import concourse.bass as bass  # noqa: E402
import concourse.tile as tile  # noqa: E402
from concourse import mybir  # noqa: E402
from concourse.alu_op_type import AluOpType  # noqa: E402
from concourse.bass_utils import run_bass_kernel_spmd  # noqa: E402

IS = 256
NEAR, FAR = 0.1, 100.0
VIEW_ANGLE_DEG = 30.0
CAM_DIST, ELEV, AZIM = 2.732, 0.0, 90.0
EPS = 1e-9

NCORES = 8
PTILE = 128                  # pixels per tile slot (partition dim)
BH, BW = 16, 8               # pixel block shape (BH*BW == PTILE)
PAD = 8                      # face-count padding granularity
MAXPK = 504                  # max packed-item columns (one PSUM bank, 3|MAXPK)
MAXCAP = 504                 # max faces per single matmul (<= 512 bank cols)
KSPLIT = 3                   # bf16 components per fp32 coefficient
K = 3 * KSPLIT               # matmul contraction dim
DUMMY_XY = -4.0              # off-screen coord for padding pixels

_prog_cache: dict = {}


class LeanTileContext(tile.TileContext):
    """TileContext without the end-of-kernel drain/barrier/sem-clear.

    The stock ending (drain + barriers + per-sem clears) costs ~10us.  The
    runtime's NEFF-end quiesce already waits for outstanding DMA, and
    semaphores are reinitialized on load, so correctness holds across
    repeated executions (verified on hardware).
    """

    def _drain_and_barrier(self, tick_clock, wait_clock):
        popped = self.nc._tile_sem_poison_stack.pop()
        assert popped is self._sem_poison


def _camera_transform(v: np.ndarray) -> np.ndarray:
    """Replicate reference's look_at + perspective in fp32. v: [V,3]."""
    e, a = np.radians(ELEV), np.radians(AZIM)
    eye = np.array(
        [
            CAM_DIST * np.cos(e) * np.sin(a),
            CAM_DIST * np.sin(e),
            -CAM_DIST * np.cos(e) * np.cos(a),
        ],
        dtype=np.float32,
    )
    at = np.zeros(3, np.float32)
    up = np.array([0.0, 1.0, 0.0], np.float32)
    z = at - eye
    z = (z / np.linalg.norm(z)).astype(np.float32)
    x = np.cross(up, z)
    x = (x / np.linalg.norm(x)).astype(np.float32)
    y = np.cross(z, x)
    y = (y / np.linalg.norm(y)).astype(np.float32)
    R = np.stack([x, y, z]).astype(np.float32)
    vc = ((v - eye) @ R.T).astype(np.float32)
    w = np.float32(np.tan(np.radians(VIEW_ANGLE_DEG)))
    zc = vc[:, 2]
    return np.stack([vc[:, 0] / (zc * w), vc[:, 1] / (zc * w), zc], -1).astype(
        np.float32
    )


def _face_coefficients(fv: np.ndarray):
    """Affine coefficients per map: returns (coeffs [nmaps,3,F] f32,
    valid [F] bool, nmaps)."""
    F = fv.shape[0]
    x0, x1, x2 = fv[:, 0, 0], fv[:, 1, 0], fv[:, 2, 0]
    y0, y1, y2 = fv[:, 0, 1], fv[:, 1, 1], fv[:, 2, 1]
    z0, z1, z2 = fv[:, 0, 2], fv[:, 1, 2], fv[:, 2, 2]

    denom = (y1 - y2) * (x0 - x2) + (x2 - x1) * (y0 - y2)
    valid = (np.abs(denom) > EPS) & np.all(np.isfinite(fv.reshape(F, -1)), -1)
    d = np.where(valid, denom, np.float32(1.0)).astype(np.float32)

    a0 = (y1 - y2) / d
    b0 = (x2 - x1) / d
    c0 = -(a0 * x2 + b0 * y2)
    a1 = (y2 - y0) / d
    b1 = (x0 - x2) / d
    c1 = -(a1 * x2 + b1 * y2)
    a2 = -(a0 + a1)
    b2 = -(b0 + b1)
    c2 = np.float32(1.0) - c0 - c1

    # Depth redundancy: for an interior pixel the perspective-correct depth
    # is a harmonic mean of vertex z's, hence inside (NEAR, FAR) whenever
    # all (valid-face) vertex z's are.
    z_valid = fv[valid][:, :, 2] if valid.any() else np.array([[1.0]])
    depth_safe = bool(
        np.all((z_valid > NEAR * 1.0001) & (z_valid < FAR * 0.9999)))

    maps = [(a0, b0, c0), (a1, b1, c1), (a2, b2, c2)]
    if not depth_safe:
        iz0 = np.float32(1.0) / z0
        iz1 = np.float32(1.0) / z1
        iz2 = np.float32(1.0) / z2
        az = a0 * iz0 + a1 * iz1 + a2 * iz2
        bz = b0 * iz0 + b1 * iz1 + b2 * iz2
        cz = c0 * iz0 + c1 * iz1 + c2 * iz2
        maps.append((az, bz, cz - np.float32(1.0 / FAR)))
        maps.append((-az, -bz, np.float32(1.0 / NEAR) - cz))

    nmaps = len(maps)
    coeffs = np.empty((nmaps, 3, F), np.float32)
    for m, (a, b, c) in enumerate(maps):
        bad = ~(valid & np.isfinite(a) & np.isfinite(b) & np.isfinite(c))
        coeffs[m, 0] = np.where(bad, np.float32(0.0), a)
        coeffs[m, 1] = np.where(bad, np.float32(0.0), b)
        coeffs[m, 2] = np.where(bad, np.float32(-1.0), c)
    return coeffs, valid, nmaps


def _split_bf16(v: np.ndarray) -> list[np.ndarray]:
    """Split fp32 array into KSPLIT bf16 components summing to ~v (2^-25)."""
    parts = []
    rem = v.astype(np.float32)
    for _ in range(KSPLIT):
        p = rem.astype(ml_dtypes.bfloat16)
        parts.append(p)
        rem = (rem - p.astype(np.float32)).astype(np.float32)
    return parts


def _make_items(nmaps: int, caps: tuple[int, ...]):
    """Group slots (face capacities, desc order) into device work items.

    ("p", cap, S, j0): S slots j0..j0+S-1, each padded to cap columns;
        one matmul per slot into a shared PSUM bank (nmaps*cap*S <= MAXPK).
    ("b", cap, j, chunks): slot j processed as len(chunks) chunks, each
        chunk a per-map matmul group into a 3-bank tile (cap <= MAXCAP
        per chunk); chunks[i] is the chunk's capacity.
    """
    items = []
    NT = len(caps)
    maxpk = (MAXPK // nmaps // PAD) * PAD * nmaps  # pack budget, PAD-aligned
    j = 0
    while j < NT:
        c = caps[j]
        if nmaps * c <= maxpk:
            S = 1
            while j + S < NT and nmaps * c * (S + 1) <= maxpk:
                S += 1
            items.append(("p", c, S, j))
            j += S
        else:
            nch = int(np.ceil(c / MAXCAP))
            ch = int(np.ceil(c / nch / PAD)) * PAD
            chunks = []
            left = c
            while left > 0:
                chunks.append(min(ch, max(PAD, left)))
                left -= chunks[-1]
            items.append(("b", c, j, tuple(chunks)))
            j += 1
    return items


def _make_schedule(vertices, image_ref, faces):
    """Host planning: prune + block + cull + deal.

    Returns (in_maps, nmaps, caps, items, host_extra)."""
    v = np.asarray(vertices, np.float32)[0]
    f = np.asarray(faces)[0].astype(np.int64)
    img = np.asarray(image_ref, np.float32)[0]
    img_flat = img.reshape(-1)

    vp = _camera_transform(v)
    fv = vp[f]                                    # [F,3,3]
    coeffs, valid, nmaps = _face_coefficients(fv)
    F = fv.shape[0]

    i = np.arange(IS, dtype=np.float32)
    xcol = (2.0 * i + 1.0 - IS) / IS
    yrow = (2.0 * (IS - 1.0 - i) + 1.0 - IS) / IS   # decreasing in row
    marg = np.float32(2.0 / IS)                     # one-pixel margin

    vi = np.where(valid)[0]
    if len(vi):
        fx = fv[:, :, 0]
        fy = fv[:, :, 1]
        fxmin, fxmax = fx.min(1), fx.max(1)
        fymin, fymax = fy.min(1), fy.max(1)
        gxmin, gxmax = fxmin[vi].min(), fxmax[vi].max()
        gymin, gymax = fymin[vi].min(), fymax[vi].max()
        rows = np.where((yrow >= gymin - marg) & (yrow <= gymax + marg))[0]
        cols = np.where((xcol >= gxmin - marg) & (xcol <= gxmax + marg))[0]
    else:
        rows = cols = np.array([], np.int64)

    A = coeffs[:, 0, :]                           # [nmaps, F]
    B = coeffs[:, 1, :]
    C = coeffs[:, 2, :]

    blocks = []        # active: (count, face_idx, pixel_idx)
    covered_extra = np.float32(0.0)
    handled = np.zeros(IS * IS, bool)  # covered-block pixels (host-summed)
    if len(rows) and len(cols):
        r0, r1 = int(rows.min()), int(rows.max()) + 1
        c0, c1 = int(cols.min()), int(cols.max()) + 1
        for rr in range(r0, r1, BH):
            for cc in range(c0, c1, BW):
                rr2, cc2 = min(rr + BH, r1), min(cc + BW, c1)
                ylo, yhi = yrow[rr2 - 1] - marg, yrow[rr] + marg
                xlo, xhi = xcol[cc] - marg, xcol[cc2 - 1] + marg
                inter = valid & (fymax >= ylo) & (fymin <= yhi) \
                    & (fxmax >= xlo) & (fxmin <= xhi)
                fl = np.where(inter)[0]
                if not len(fl):
                    continue          # pixels stay unassigned -> host ref^2
                rgrid, cgrid = np.meshgrid(np.arange(rr, rr2),
                                           np.arange(cc, cc2), indexing="ij")
                px = (rgrid * IS + cgrid).reshape(-1)
                # full-coverage cull: some face has every affine map
                # strictly positive at all 4 corner pixel centers
                cx = np.array([xcol[cc], xcol[cc2 - 1]], np.float32)
                cy = np.array([yrow[rr], yrow[rr2 - 1]], np.float32)
                CX, CY = np.meshgrid(cx, cy)
                P0, P1 = CX.ravel(), CY.ravel()   # [4]
                W = (A[:, fl, None] * P0[None, None, :]
                     + B[:, fl, None] * P1[None, None, :]
                     + C[:, fl, None])            # [nmaps, nf, 4]
                if bool(np.any((W > 1e-6).all(axis=(0, 2)))):
                    covered_extra += np.sum(
                        np.square(np.float32(1.0) - img_flat[px]),
                        dtype=np.float32)
                    handled[px] = True
                    continue
                blocks.append((len(fl), fl, px))

    if not blocks:
        blocks = [(0, np.array([], np.int64), np.array([], np.int64))]

    blocks.sort(key=lambda b: -b[0])
    NT = (len(blocks) + NCORES - 1) // NCORES
    empty = (0, np.array([], np.int64), np.array([], np.int64))
    while len(blocks) < NT * NCORES:
        blocks.append(empty)

    caps = []
    for j in range(NT):
        grp = blocks[NCORES * j:NCORES * (j + 1)]
        caps.append(max(PAD, int(np.ceil(max(b[0] for b in grp) / PAD)) * PAD))
    caps = tuple(caps)
    items = _make_items(nmaps, caps)

    # per-slot column count in the coef stream (incl. padding), layouts:
    #   "p": per slot s: [m0 x cap | m1 x cap | m2 x cap]   (slot-major)
    #   "b": per chunk:  [m0 x ch | m1 x ch | m2 x ch]
    CW = 0
    for it in items:
        if it[0] == "p":
            CW += nmaps * it[1] * it[2]
        else:
            CW += nmaps * sum(it[3])

    # coefficient splits with a trailing dummy column (index F)
    csp = np.empty((nmaps, 3, KSPLIT, F + 1), ml_dtypes.bfloat16)
    for m in range(nmaps):
        for j3 in range(3):
            col = np.concatenate(
                [coeffs[m, j3],
                 [np.float32(-1.0 if j3 == 2 else 0.0)]])
            for s, part in enumerate(_split_bf16(col)):
                csp[m, j3, s] = part

    PIXW = NT * PTILE
    assigned = handled
    in_maps = []
    for k in range(NCORES):
        pix = np.full((K, PIXW), np.float32(DUMMY_XY), np.float32)
        ref = np.zeros((PTILE, NT), np.float32)
        coef = np.empty((K, CW), ml_dtypes.bfloat16)
        # pixels + ref per slot
        slot_fidx = []
        for j in range(NT):
            cnt, fl, px = blocks[NCORES * j + k]
            npx = len(px)
            if npx:
                lane_x = xcol[px % IS]
                lane_y = yrow[px // IS]
                for s in range(KSPLIT):
                    pix[s * 3 + 0, j * PTILE:j * PTILE + npx] = lane_x
                    pix[s * 3 + 1, j * PTILE:j * PTILE + npx] = lane_y
                ref[:npx, j] = img_flat[px]
                assigned[px] = True
            for s in range(KSPLIT):
                pix[s * 3 + 2, j * PTILE:(j + 1) * PTILE] = 1.0
            slot_fidx.append((cnt, fl))
        # coefficients per item
        cb = 0
        for it in items:
            if it[0] == "p":
                _, cap, S, j0 = it
                for s in range(S):
                    cnt, fl = slot_fidx[j0 + s]
                    fidx = np.full(cap, F, np.int64)
                    fidx[:cnt] = fl
                    for m in range(nmaps):
                        for ks in range(KSPLIT):
                            for j3 in range(3):
                                coef[ks * 3 + j3,
                                     cb + m * cap:cb + (m + 1) * cap] = \
                                    csp[m, j3, ks][fidx]
                    cb += nmaps * cap
            else:
                _, cap, j, chunks = it
                cnt, fl = slot_fidx[j]
                fidx_all = np.full(sum(chunks), F, np.int64)
                fidx_all[:cnt] = fl
                pos = 0
                for ch in chunks:
                    sel = fidx_all[pos:pos + ch]
                    for m in range(nmaps):
                        for ks in range(KSPLIT):
                            for j3 in range(3):
                                coef[ks * 3 + j3,
                                     cb + m * ch:cb + (m + 1) * ch] = \
                                    csp[m, j3, ks][sel]
                    cb += nmaps * ch
                    pos += ch
        assert cb == CW
        in_maps.append({
            "coef": np.concatenate(
                [pix.astype(ml_dtypes.bfloat16), coef], axis=1),
            "ref": ref,
        })

    host_extra = float(np.sum(np.square(img_flat[~assigned]),
                              dtype=np.float32) + covered_extra)
    return in_maps, nmaps, caps, items, host_extra


def _build_program(nmaps: int, caps, items) -> bass.Bass:
    NT = len(caps)
    PIXW = NT * PTILE
    CW = 0
    for it in items:
        CW += nmaps * (it[1] * it[2] if it[0] == "p" else sum(it[3]))
    # accumulator columns: one per slot + one per extra chunk
    nextra = sum(len(it[3]) - 1 for it in items if it[0] == "b")
    NACC = NT + max(nextra, 1)

    nc = bacc.Bacc()
    coef_d = nc.dram_tensor("coef", [K, PIXW + CW], mybir.dt.bfloat16,
                            kind="ExternalInput")
    ref_d = nc.dram_tensor("ref", [PTILE, NT], mybir.dt.float32,
                           kind="ExternalInput")
    out_d = nc.dram_tensor("out", [PTILE, 1], mybir.dt.float32,
                           kind="ExternalOutput")

    with LeanTileContext(nc) as tc:
        with ExitStack() as ctx:
            const = ctx.enter_context(tc.tile_pool(name="const", bufs=1))
            total = PIXW + CW
            # part0 (sync): pixels + leading coef; part1 (gpsimd): the rest.
            cut = PIXW + (int(0.45 * CW) // 8) * 8
            cut = min(cut, total)
            cf = const.tile([K, total], mybir.dt.bfloat16)
            nc.sync.dma_start(cf[:, :cut], coef_d[:, :cut])
            if cut < total:
                nc.gpsimd.dma_start(cf[:, cut:], coef_d[:, cut:])
            ref_s = const.tile([PTILE, NT], mybir.dt.float32)
            nc.scalar.dma_start(ref_s[:], ref_d[:])
            mx = const.tile([PTILE, NACC], mybir.dt.float32)

            psum = ctx.enter_context(
                tc.tile_pool(name="psum", bufs=2, space="PSUM"))
            tmp = ctx.enter_context(tc.tile_pool(name="tmp", bufs=3))

            eidx = NT
            for it in items:
                if it[0] == "p":
                    _, cap, S, j0 = it
                    N = nmaps * cap
                    wp = psum.tile([PTILE, 512], mybir.dt.float32,
                                   tag="pk", bufs=2)
                    for s in range(S):
                        j = j0 + s
                        off = PIXW + _coef_off(nmaps, items, it) \
                            + s * N
                        nc.tensor.matmul(
                            wp[:, s * N:(s + 1) * N],
                            cf[:K, j * PTILE:(j + 1) * PTILE],
                            cf[:K, off:off + N],
                            start=True, stop=True)
                    wv = wp[:, :S * N].rearrange(
                        "p (s m b) -> p s m b", s=S, m=nmaps)
                    st = tmp.tile([PTILE, 1024], mybir.dt.bfloat16, tag="st")
                    stv = st[:, :S * 2 * cap].rearrange(
                        "p (s m b) -> p s m b", s=S, m=2)
                    nc.scalar.copy(stv, wv[:, :, 0:2, :])
                    mn = tmp.tile([PTILE, 512], mybir.dt.bfloat16, tag="mn")
                    mnv = mn[:, :S * cap].rearrange(
                        "p (s b) -> p s b", s=S)
                    nc.vector.tensor_tensor(mnv, stv[:, :, 0, :],
                                            stv[:, :, 1, :],
                                            op=AluOpType.min)
                    for m in range(2, nmaps):
                        nc.vector.tensor_tensor(mnv, mnv, wv[:, :, m, :],
                                                op=AluOpType.min)
                    nc.vector.reduce_max(mx[:, j0:j0 + S], mnv,
                                         axis=mybir.AxisListType.X)
                else:
                    _, cap, j, chunks = it
                    off = PIXW + _coef_off(nmaps, items, it)
                    for ci, ch in enumerate(chunks):
                        wp = psum.tile([PTILE, 1536], mybir.dt.float32,
                                       tag="big", bufs=2)
                        for m in range(nmaps):
                            nc.tensor.matmul(
                                wp[:, 512 * m:512 * m + ch],
                                cf[:K, j * PTILE:(j + 1) * PTILE],
                                cf[:K, off + m * ch:off + (m + 1) * ch],
                                start=True, stop=True)
                        off += nmaps * ch
                        st = tmp.tile([PTILE, 1024], mybir.dt.bfloat16,
                                      tag="st")
                        stv = st[:, :2 * ch].rearrange(
                            "p (m b) -> p m b", m=2)
                        nc.scalar.copy(
                            stv,
                            wp[:].rearrange(
                                "p (m b) -> p m b", m=3)[:, 0:2, :ch])
                        mn = tmp.tile([PTILE, 512], mybir.dt.bfloat16,
                                      tag="mn")
                        nc.vector.tensor_tensor(mn[:, :ch], stv[:, 0, :],
                                                stv[:, 1, :],
                                                op=AluOpType.min)
                        for m in range(2, nmaps):
                            nc.vector.tensor_tensor(
                                mn[:, :ch], mn[:, :ch],
                                wp[:, 512 * m:512 * m + ch],
                                op=AluOpType.min)
                        dst = mx[:, j:j + 1] if ci == 0 \
                            else mx[:, eidx:eidx + 1]
                        nc.vector.reduce_max(dst, mn[:, :ch],
                                             axis=mybir.AxisListType.X)
                        if ci > 0:
                            nc.vector.tensor_tensor(
                                mx[:, j:j + 1], mx[:, j:j + 1],
                                mx[:, eidx:eidx + 1], op=AluOpType.max)
                            eidx += 1

            # diff = (mx > 0 ? 1.0 : 0.0) - ref ; out = rowsum(diff^2)
            diff = const.tile([PTILE, NT], mybir.dt.float32)
            nc.vector.scalar_tensor_tensor(
                out=diff[:], in0=mx[:, :NT], scalar=0.0, in1=ref_s[:],
                op0=AluOpType.is_gt, op1=AluOpType.subtract)
            sq = const.tile([PTILE, NT], mybir.dt.float32)
            nc.vector.tensor_tensor(sq[:], diff[:], diff[:],
                                    op=AluOpType.mult)
            losscol = const.tile([PTILE, 1], mybir.dt.float32)
            nc.vector.reduce_sum(losscol[:], sq[:],
                                 axis=mybir.AxisListType.X)
            nc.sync.dma_start(out_d[:], losscol[:])
    nc.compile()
    return nc


def _coef_off(nmaps: int, items, target) -> int:
    off = 0
    for it in items:
        if it is target:
            return off
        off += nmaps * (it[1] * it[2] if it[0] == "p" else sum(it[3]))
    raise KeyError(target)


def run_sharded(vertices, image_ref, faces, trace=False, **spmd_kwargs):
    """Runs the SPMD kernel on 8 cores; returns (loss, BassKernelResults)."""
    in_maps, nmaps, caps, items, host_extra = _make_schedule(
        vertices, image_ref, faces)
    key = (nmaps, caps, tuple(items))
    if key not in _prog_cache:
        _prog_cache[key] = _build_program(nmaps, caps, items)
    nc = _prog_cache[key]
    results = run_bass_kernel_spmd(
        nc, in_maps, core_ids=list(range(NCORES)), trace=trace, **spmd_kwargs)
    partials = np.stack([r["out"].reshape(-1) for r in results.results])
    loss = np.float32(partials.astype(np.float32).sum(dtype=np.float32)
                      + np.float32(host_extra))
    return loss, results


def kernel(vertices: np.ndarray, image_ref: np.ndarray,
           faces: np.ndarray) -> np.ndarray:
    loss, _ = run_sharded(vertices, image_ref, faces, trace=False)
    return np.asarray(loss, dtype=np.float32)


# revision 7
# speedup vs baseline: 1.8289x; 1.0198x over previous
"""Trainium2 Bass kernel for the neural-renderer silhouette MSE loss.

Reference computation: project 512 vertices, gather 1024 triangle faces,
rasterize a 256x256 silhouette (a pixel is covered iff it lies strictly
inside some valid face and the perspective-correct depth is in (NEAR, FAR)),
then return sum((sil - image_ref)^2).

Reformulation: each barycentric weight w_i of face f is an *affine* function
of the pixel NDC coords, w_i = a_i*x + b_i*y + c_i, so
    covered(p) = max_f min_i w_i(p, f) > 0.
The depth test is provably redundant when every camera-space vertex z lies
inside (NEAR, FAR); otherwise two extra affine maps are appended to the min.

Work pruning (host-side, exact):
  - A pixel strictly outside the global face bounding box can never be
    covered; its loss term ref^2 is summed on the host.
  - The in-bbox area is cut into 16x8-pixel blocks (= one 128-lane tile
    each). Each block only needs faces whose bbox overlaps it.
  - A block that lies fully inside a single valid face (all affine maps
    strictly positive at the block's 4 corner pixel centers, hence by
    linearity at every interior pixel center) is fully covered: the host
    adds sum((1-ref)^2) for it and the device never sees it.  This culls
    the (large) silhouette interior, leaving only boundary-ish blocks.
  - Remaining active blocks are sorted by face count and snake-dealt to
    the 8 cores, so all cores run an identical slot schedule (SPMD) whose
    per-slot face capacity is the max count in the 8-block group (pad 8).

Device (SPMD, one program on 8 cores; schedule baked at build time):
  - PE: per slot one K=9 bf16 matmul (lhsT = pixel matrix [9, 128],
    rhs = coefficients [9, 3*cap]) -> PSUM.  Small slots are packed S per
    PSUM bank (3*cap*S <= 504); large slots use a 3-bank tile with one
    matmul per affine map.  Each fp32 coefficient is split into 3 bf16
    components (exact to ~2^-25); pixel coords are exactly representable
    in bf16, so fp32 PSUM accumulation reproduces fp32 affine values
    essentially exactly.
  - ACT: stages maps 0,1 PSUM->SBUF as bf16 in one strided copy (the DVE
    reads at most one PSUM operand per instruction; sign-exact suffices).
  - DVE: min(w0,w1) at 2x (both SBUF bf16), min(.,w2) vs PSUM, grouped
    reduce_max over faces -> per-slot maxima; epilogue computes
    sum(((mx>0) - ref)^2) per partition row.
  - No end-of-kernel drain/barrier/sem-clear: the NEFF-end quiesce covers
    DMA completion, and the runtime reinitializes semaphores per load
    (verified empirically over repeated executions).
  - Host: sums 8x128 partials + the culled blocks' closed-form terms.
"""

import os
import sys
from contextlib import ExitStack

import numpy as np

for _p in (
    "/opt/trn_rl_repo",
    "/root/.axon_site",
    "/root/.axon_site/_ro/trn_rl_repo",
    "/root/.axon_site/_ro/pypackages",
):
    if os.path.isdir(_p) and _p not in sys.path:
        sys.path.append(_p)

import ml_dtypes  # noqa: E402

import concourse.bacc as bacc  # noqa: E402
import concourse.bass as bass  # noqa: E402
import concourse.tile as tile  # noqa: E402
from concourse import mybir  # noqa: E402
from concourse.alu_op_type import AluOpType  # noqa: E402
from concourse.bass_utils import run_bass_kernel_spmd  # noqa: E402

IS = 256
NEAR, FAR = 0.1, 100.0
VIEW_ANGLE_DEG = 30.0
CAM_DIST, ELEV, AZIM = 2.732, 0.0, 90.0
EPS = 1e-9

NCORES = 8
PTILE = 128                  # pixels per tile slot (partition dim)
BH, BW = 16, 8               # pixel block shape (BH*BW == PTILE)
PAD = 8                      # face-count padding granularity
MAXPK = 504                  # max packed-item columns (one PSUM bank, 3|MAXPK)
MAXCAP = 504                 # max faces per single matmul (<= 512 bank cols)
KSPLIT = 3                   # bf16 components per fp32 coefficient
K = 3 * KSPLIT               # matmul contraction dim
DUMMY_XY = -4.0              # off-screen coord for padding pixels

_prog_cache: dict = {}


class LeanTileContext(tile.TileContext):
    """TileContext without the end-of-kernel drain/barrier/sem-clear.

    The stock ending (drain + barriers + per-sem clears) costs ~10us.  The
    runtime's NEFF-end quiesce already waits for outstanding DMA, and
    semaphores are reinitialized on load, so correctness holds across
    repeated executions (verified on hardware).
    """

    def _drain_and_barrier(self, tick_clock, wait_clock):
        popped = self.nc._tile_sem_poison_stack.pop()
        assert popped is self._sem_poison


def _camera_transform(v: np.ndarray) -> np.ndarray:
    """Replicate reference's look_at + perspective in fp32. v: [V,3]."""
    e, a = np.radians(ELEV), np.radians(AZIM)
    eye = np.array(
        [
            CAM_DIST * np.cos(e) * np.sin(a),
            CAM_DIST * np.sin(e),
            -CAM_DIST * np.cos(e) * np.cos(a),
        ],
        dtype=np.float32,
    )
    at = np.zeros(3, np.float32)
    up = np.array([0.0, 1.0, 0.0], np.float32)
    z = at - eye
    z = (z / np.linalg.norm(z)).astype(np.float32)
    x = np.cross(up, z)
    x = (x / np.linalg.norm(x)).astype(np.float32)
    y = np.cross(z, x)
    y = (y / np.linalg.norm(y)).astype(np.float32)
    R = np.stack([x, y, z]).astype(np.float32)
    vc = ((v - eye) @ R.T).astype(np.float32)
    w = np.float32(np.tan(np.radians(VIEW_ANGLE_DEG)))
    zc = vc[:, 2]
    return np.stack([vc[:, 0] / (zc * w), vc[:, 1] / (zc * w), zc], -1).astype(
        np.float32
    )


def _face_coefficients(fv: np.ndarray):
    """Affine coefficients per map: returns (coeffs [nmaps,3,F] f32,
    valid [F] bool, nmaps)."""
    F = fv.shape[0]
    x0, x1, x2 = fv[:, 0, 0], fv[:, 1, 0], fv[:, 2, 0]
    y0, y1, y2 = fv[:, 0, 1], fv[:, 1, 1], fv[:, 2, 1]
    z0, z1, z2 = fv[:, 0, 2], fv[:, 1, 2], fv[:, 2, 2]

    denom = (y1 - y2) * (x0 - x2) + (x2 - x1) * (y0 - y2)
    valid = (np.abs(denom) > EPS) & np.all(np.isfinite(fv.reshape(F, -1)), -1)
    d = np.where(valid, denom, np.float32(1.0)).astype(np.float32)

    a0 = (y1 - y2) / d
    b0 = (x2 - x1) / d
    c0 = -(a0 * x2 + b0 * y2)
    a1 = (y2 - y0) / d
    b1 = (x0 - x2) / d
    c1 = -(a1 * x2 + b1 * y2)
    a2 = -(a0 + a1)
    b2 = -(b0 + b1)
    c2 = np.float32(1.0) - c0 - c1

    # Depth redundancy: for an interior pixel the perspective-correct depth
    # is a harmonic mean of vertex z's, hence inside (NEAR, FAR) whenever
    # all (valid-face) vertex z's are.
    z_valid = fv[valid][:, :, 2] if valid.any() else np.array([[1.0]])
    depth_safe = bool(
        np.all((z_valid > NEAR * 1.0001) & (z_valid < FAR * 0.9999)))

    maps = [(a0, b0, c0), (a1, b1, c1), (a2, b2, c2)]
    if not depth_safe:
        iz0 = np.float32(1.0) / z0
        iz1 = np.float32(1.0) / z1
        iz2 = np.float32(1.0) / z2
        az = a0 * iz0 + a1 * iz1 + a2 * iz2
        bz = b0 * iz0 + b1 * iz1 + b2 * iz2
        cz = c0 * iz0 + c1 * iz1 + c2 * iz2
        maps.append((az, bz, cz - np.float32(1.0 / FAR)))
        maps.append((-az, -bz, np.float32(1.0 / NEAR) - cz))

    nmaps = len(maps)
    coeffs = np.empty((nmaps, 3, F), np.float32)
    for m, (a, b, c) in enumerate(maps):
        bad = ~(valid & np.isfinite(a) & np.isfinite(b) & np.isfinite(c))
        coeffs[m, 0] = np.where(bad, np.float32(0.0), a)
        coeffs[m, 1] = np.where(bad, np.float32(0.0), b)
        coeffs[m, 2] = np.where(bad, np.float32(-1.0), c)
    return coeffs, valid, nmaps


def _split_bf16(v: np.ndarray) -> list[np.ndarray]:
    """Split fp32 array into KSPLIT bf16 components summing to ~v (2^-25)."""
    parts = []
    rem = v.astype(np.float32)
    for _ in range(KSPLIT):
        p = rem.astype(ml_dtypes.bfloat16)
        parts.append(p)
        rem = (rem - p.astype(np.float32)).astype(np.float32)
    return parts


def _make_items(nmaps: int, caps: tuple[int, ...]):
    """Group slots (face capacities, desc order) into device work items.

    ("p", cap, S, j0): S slots j0..j0+S-1, each padded to cap columns;
        one matmul per slot into a shared PSUM bank (nmaps*cap*S <= MAXPK).
    ("b", cap, j, chunks): slot j processed as len(chunks) chunks, each
        chunk a per-map matmul group into a 3-bank tile (cap <= MAXCAP
        per chunk); chunks[i] is the chunk's capacity.
    """
    items = []
    NT = len(caps)
    maxpk = (MAXPK // nmaps // PAD) * PAD * nmaps  # pack budget, PAD-aligned
    j = 0
    while j < NT:
        c = caps[j]
        if nmaps * c <= maxpk:
            S = 1
            while j + S < NT and nmaps * c * (S + 1) <= maxpk:
                S += 1
            items.append(("p", c, S, j))
            j += S
        else:
            nch = int(np.ceil(c / MAXCAP))
            ch = int(np.ceil(c / nch / PAD)) * PAD
            chunks = []
            left = c
            while left > 0:
                chunks.append(min(ch, max(PAD, left)))
                left -= chunks[-1]
            items.append(("b", c, j, tuple(chunks)))
            j += 1
    return items


def _make_schedule(vertices, image_ref, faces):
    """Host planning: prune + block + cull + deal.

    Returns (in_maps, nmaps, caps, items, host_extra)."""
    v = np.asarray(vertices, np.float32)[0]
    f = np.asarray(faces)[0].astype(np.int64)
    img = np.asarray(image_ref, np.float32)[0]
    img_flat = img.reshape(-1)

    vp = _camera_transform(v)
    fv = vp[f]                                    # [F,3,3]
    coeffs, valid, nmaps = _face_coefficients(fv)
    F = fv.shape[0]

    i = np.arange(IS, dtype=np.float32)
    xcol = (2.0 * i + 1.0 - IS) / IS
    yrow = (2.0 * (IS - 1.0 - i) + 1.0 - IS) / IS   # decreasing in row
    marg = np.float32(2.0 / IS)                     # one-pixel margin

    vi = np.where(valid)[0]
    if len(vi):
        fx = fv[:, :, 0]
        fy = fv[:, :, 1]
        fxmin, fxmax = fx.min(1), fx.max(1)
        fymin, fymax = fy.min(1), fy.max(1)
        gxmin, gxmax = fxmin[vi].min(), fxmax[vi].max()
        gymin, gymax = fymin[vi].min(), fymax[vi].max()
        rows = np.where((yrow >= gymin - marg) & (yrow <= gymax + marg))[0]
        cols = np.where((xcol >= gxmin - marg) & (xcol <= gxmax + marg))[0]
    else:
        rows = cols = np.array([], np.int64)

    A = coeffs[:, 0, :]                           # [nmaps, F]
    B = coeffs[:, 1, :]
    C = coeffs[:, 2, :]

    blocks = []        # active: (count, face_idx, pixel_idx)
    covered_extra = np.float32(0.0)
    handled = np.zeros(IS * IS, bool)  # covered-block pixels (host-summed)
    if len(rows) and len(cols):
        r0, r1 = int(rows.min()), int(rows.max()) + 1
        c0, c1 = int(cols.min()), int(cols.max()) + 1
        for rr in range(r0, r1, BH):
            for cc in range(c0, c1, BW):
                rr2, cc2 = min(rr + BH, r1), min(cc + BW, c1)
                ylo, yhi = yrow[rr2 - 1] - marg, yrow[rr] + marg
                xlo, xhi = xcol[cc] - marg, xcol[cc2 - 1] + marg
                inter = valid & (fymax >= ylo) & (fymin <= yhi) \
                    & (fxmax >= xlo) & (fxmin <= xhi)
                fl = np.where(inter)[0]
                if not len(fl):
                    continue          # pixels stay unassigned -> host ref^2
                rgrid, cgrid = np.meshgrid(np.arange(rr, rr2),
                                           np.arange(cc, cc2), indexing="ij")
                px = (rgrid * IS + cgrid).reshape(-1)
                # full-coverage cull: some face has every affine map
                # strictly positive at all 4 corner pixel centers
                cx = np.array([xcol[cc], xcol[cc2 - 1]], np.float32)
                cy = np.array([yrow[rr], yrow[rr2 - 1]], np.float32)
                CX, CY = np.meshgrid(cx, cy)
                P0, P1 = CX.ravel(), CY.ravel()   # [4]
                W = (A[:, fl, None] * P0[None, None, :]
                     + B[:, fl, None] * P1[None, None, :]
                     + C[:, fl, None])            # [nmaps, nf, 4]
                if bool(np.any((W > 1e-6).all(axis=(0, 2)))):
                    covered_extra += np.sum(
                        np.square(np.float32(1.0) - img_flat[px]),
                        dtype=np.float32)
                    handled[px] = True
                    continue
                blocks.append((len(fl), fl, px))

    if not blocks:
        blocks = [(0, np.array([], np.int64), np.array([], np.int64))]

    blocks.sort(key=lambda b: -b[0])
    NT = (len(blocks) + NCORES - 1) // NCORES
    empty = (0, np.array([], np.int64), np.array([], np.int64))
    while len(blocks) < NT * NCORES:
        blocks.append(empty)

    caps = []
    for j in range(NT):
        grp = blocks[NCORES * j:NCORES * (j + 1)]
        caps.append(max(PAD, int(np.ceil(max(b[0] for b in grp) / PAD)) * PAD))
    caps = tuple(caps)
    items = _make_items(nmaps, caps)

    # per-slot column count in the coef stream (incl. padding), layouts:
    #   "p": per slot s: [m0 x cap | m1 x cap | m2 x cap]   (slot-major)
    #   "b": per chunk:  [m0 x ch | m1 x ch | m2 x ch]
    CW = 0
    for it in items:
        if it[0] == "p":
            CW += nmaps * it[1] * it[2]
        else:
            CW += nmaps * sum(it[3])

    # coefficient splits with a trailing dummy column (index F)
    csp = np.empty((nmaps, 3, KSPLIT, F + 1), ml_dtypes.bfloat16)
    for m in range(nmaps):
        for j3 in range(3):
            col = np.concatenate(
                [coeffs[m, j3],
                 [np.float32(-1.0 if j3 == 2 else 0.0)]])
            for s, part in enumerate(_split_bf16(col)):
                csp[m, j3, s] = part

    PIXW = NT * PTILE
    assigned = handled
    in_maps = []
    for k in range(NCORES):
        pix = np.full((K, PIXW), np.float32(DUMMY_XY), np.float32)
        ref = np.zeros((PTILE, NT), np.float32)
        coef = np.empty((K, CW), ml_dtypes.bfloat16)
        # pixels + ref per slot
        slot_fidx = []
        for j in range(NT):
            cnt, fl, px = blocks[NCORES * j + k]
            npx = len(px)
            if npx:
                lane_x = xcol[px % IS]
                lane_y = yrow[px // IS]
                for s in range(KSPLIT):
                    pix[s * 3 + 0, j * PTILE:j * PTILE + npx] = lane_x
                    pix[s * 3 + 1, j * PTILE:j * PTILE + npx] = lane_y
                ref[:npx, j] = img_flat[px]
                assigned[px] = True
            for s in range(KSPLIT):
                pix[s * 3 + 2, j * PTILE:(j + 1) * PTILE] = 1.0
            slot_fidx.append((cnt, fl))
        # coefficients per item
        cb = 0
        for it in items:
            if it[0] == "p":
                _, cap, S, j0 = it
                for s in range(S):
                    cnt, fl = slot_fidx[j0 + s]
                    fidx = np.full(cap, F, np.int64)
                    fidx[:cnt] = fl
                    for m in range(nmaps):
                        for ks in range(KSPLIT):
                            for j3 in range(3):
                                coef[ks * 3 + j3,
                                     cb + m * cap:cb + (m + 1) * cap] = \
                                    csp[m, j3, ks][fidx]
                    cb += nmaps * cap
            else:
                _, cap, j, chunks = it
                cnt, fl = slot_fidx[j]
                fidx_all = np.full(sum(chunks), F, np.int64)
                fidx_all[:cnt] = fl
                pos = 0
                for ch in chunks:
                    sel = fidx_all[pos:pos + ch]
                    for m in range(nmaps):
                        for ks in range(KSPLIT):
                            for j3 in range(3):
                                coef[ks * 3 + j3,
                                     cb + m * ch:cb + (m + 1) * ch] = \
                                    csp[m, j3, ks][sel]
                    cb += nmaps * ch
                    pos += ch
        assert cb == CW
        in_maps.append({
            "coef": np.concatenate(
                [pix.astype(ml_dtypes.bfloat16), coef], axis=1),
            "ref": ref,
        })

    host_extra = float(np.sum(np.square(img_flat[~assigned]),
                              dtype=np.float32) + covered_extra)
    return in_maps, nmaps, caps, items, host_extra


def _build_program(nmaps: int, caps, items) -> bass.Bass:
    NT = len(caps)
    PIXW = NT * PTILE
    CW = 0
    for it in items:
        CW += nmaps * (it[1] * it[2] if it[0] == "p" else sum(it[3]))
    # accumulator columns: one per slot + one per extra chunk
    nextra = sum(len(it[3]) - 1 for it in items if it[0] == "b")
    NACC = NT + max(nextra, 1)

    nc = bacc.Bacc()
    coef_d = nc.dram_tensor("coef", [K, PIXW + CW], mybir.dt.bfloat16,
                            kind="ExternalInput")
    ref_d = nc.dram_tensor("ref", [PTILE, NT], mybir.dt.float32,
                           kind="ExternalInput")
    out_d = nc.dram_tensor("out", [PTILE, 1], mybir.dt.float32,
                           kind="ExternalOutput")

    with LeanTileContext(nc) as tc:
        with ExitStack() as ctx:
            const = ctx.enter_context(tc.tile_pool(name="const", bufs=1))
            total = PIXW + CW
            # part0 (sync): pixels + leading items' coef; part1: the rest.
            first_cols = 0
            for it in items[:2]:
                first_cols += nmaps * (it[1] * it[2] if it[0] == "p"
                                       else sum(it[3]))
            cut = min(PIXW + first_cols, total)
            cf = const.tile([K, total], mybir.dt.bfloat16)
            nc.sync.dma_start(cf[:, :cut], coef_d[:, :cut])
            if cut < total:
                nc.sync.dma_start(cf[:, cut:], coef_d[:, cut:])
            ref_s = const.tile([PTILE, NT], mybir.dt.float32)
            nc.scalar.dma_start(ref_s[:], ref_d[:])
            mx = const.tile([PTILE, NACC], mybir.dt.float32)

            psum = ctx.enter_context(
                tc.tile_pool(name="psum", bufs=2, space="PSUM"))
            tmp = ctx.enter_context(tc.tile_pool(name="tmp", bufs=3))

            eidx = NT
            for it in items:
                if it[0] == "p":
                    _, cap, S, j0 = it
                    N = nmaps * cap
                    wp = psum.tile([PTILE, 512], mybir.dt.float32,
                                   tag="pk", bufs=2)
                    for s in range(S):
                        j = j0 + s
                        off = PIXW + _coef_off(nmaps, items, it) \
                            + s * N
                        nc.tensor.matmul(
                            wp[:, s * N:(s + 1) * N],
                            cf[:K, j * PTILE:(j + 1) * PTILE],
                            cf[:K, off:off + N],
                            start=True, stop=True)
                    wv = wp[:, :S * N].rearrange(
                        "p (s m b) -> p s m b", s=S, m=nmaps)
                    st = tmp.tile([PTILE, 1024], mybir.dt.bfloat16, tag="st")
                    # dst layout (m, s, b): all w0 contiguous, then all w1
                    stv = st[:, :S * 2 * cap].rearrange(
                        "p (m s b) -> p s m b", m=2, s=S)
                    nc.scalar.copy(stv, wv[:, :, 0:2, :])
                    mn = tmp.tile([PTILE, 512], mybir.dt.bfloat16, tag="mn")
                    mnv = mn[:, :S * cap].rearrange(
                        "p (s b) -> p s b", s=S)
                    nc.vector.tensor_tensor(mn[:, :S * cap],
                                            st[:, :S * cap],
                                            st[:, S * cap:2 * S * cap],
                                            op=AluOpType.min)
                    for m in range(2, nmaps):
                        nc.vector.tensor_tensor(mnv, mnv, wv[:, :, m, :],
                                                op=AluOpType.min)
                    nc.vector.reduce_max(mx[:, j0:j0 + S], mnv,
                                         axis=mybir.AxisListType.X)
                else:
                    _, cap, j, chunks = it
                    off = PIXW + _coef_off(nmaps, items, it)
                    for ci, ch in enumerate(chunks):
                        wp = psum.tile([PTILE, 1536], mybir.dt.float32,
                                       tag="big", bufs=2)
                        for m in range(nmaps):
                            nc.tensor.matmul(
                                wp[:, 512 * m:512 * m + ch],
                                cf[:K, j * PTILE:(j + 1) * PTILE],
                                cf[:K, off + m * ch:off + (m + 1) * ch],
                                start=True, stop=True)
                        off += nmaps * ch
                        st = tmp.tile([PTILE, 1024], mybir.dt.bfloat16,
                                      tag="st")
                        stv = st[:, :2 * ch].rearrange(
                            "p (m b) -> p m b", m=2)
                        nc.scalar.copy(
                            stv,
                            wp[:].rearrange(
                                "p (m b) -> p m b", m=3)[:, 0:2, :ch])
                        mn = tmp.tile([PTILE, 512], mybir.dt.bfloat16,
                                      tag="mn")
                        nc.vector.tensor_tensor(mn[:, :ch], stv[:, 0, :],
                                                stv[:, 1, :],
                                                op=AluOpType.min)
                        for m in range(2, nmaps):
                            nc.vector.tensor_tensor(
                                mn[:, :ch], mn[:, :ch],
                                wp[:, 512 * m:512 * m + ch],
                                op=AluOpType.min)
                        dst = mx[:, j:j + 1] if ci == 0 \
                            else mx[:, eidx:eidx + 1]
                        nc.vector.reduce_max(dst, mn[:, :ch],
                                             axis=mybir.AxisListType.X)
                        if ci > 0:
                            nc.vector.tensor_tensor(
                                mx[:, j:j + 1], mx[:, j:j + 1],
                                mx[:, eidx:eidx + 1], op=AluOpType.max)
                            eidx += 1

            # diff = (mx > 0 ? 1.0 : 0.0) - ref ; out = rowsum(diff^2)
            diff = const.tile([PTILE, NT], mybir.dt.float32)
            nc.vector.scalar_tensor_tensor(
                out=diff[:], in0=mx[:, :NT], scalar=0.0, in1=ref_s[:],
                op0=AluOpType.is_gt, op1=AluOpType.subtract)
            sq = const.tile([PTILE, NT], mybir.dt.float32)
            nc.vector.tensor_tensor(sq[:], diff[:], diff[:],
                                    op=AluOpType.mult)
            losscol = const.tile([PTILE, 1], mybir.dt.float32)
            nc.vector.reduce_sum(losscol[:], sq[:],
                                 axis=mybir.AxisListType.X)
            nc.sync.dma_start(out_d[:], losscol[:])
    nc.compile()
    return nc


def _coef_off(nmaps: int, items, target) -> int:
    off = 0
    for it in items:
        if it is target:
            return off
        off += nmaps * (it[1] * it[2] if it[0] == "p" else sum(it[3]))
    raise KeyError(target)


def run_sharded(vertices, image_ref, faces, trace=False, **spmd_kwargs):
    """Runs the SPMD kernel on 8 cores; returns (loss, BassKernelResults)."""
    in_maps, nmaps, caps, items, host_extra = _make_schedule(
        vertices, image_ref, faces)
    key = (nmaps, caps, tuple(items))
    if key not in _prog_cache:
        _prog_cache[key] = _build_program(nmaps, caps, items)
    nc = _prog_cache[key]
    results = run_bass_kernel_spmd(
        nc, in_maps, core_ids=list(range(NCORES)), trace=trace, **spmd_kwargs)
    partials = np.stack([r["out"].reshape(-1) for r in results.results])
    loss = np.float32(partials.astype(np.float32).sum(dtype=np.float32)
                      + np.float32(host_extra))
    return loss, results


def kernel(vertices: np.ndarray, image_ref: np.ndarray,
           faces: np.ndarray) -> np.ndarray:
    loss, _ = run_sharded(vertices, image_ref, faces, trace=False)
    return np.asarray(loss, dtype=np.float32)


# revision 9
# speedup vs baseline: 2.1394x; 1.1698x over previous
"""Trainium2 Bass kernel for the neural-renderer silhouette MSE loss.

Reference computation: project 512 vertices, gather 1024 triangle faces,
rasterize a 256x256 silhouette (a pixel is covered iff it lies strictly
inside some valid face and the perspective-correct depth is in (NEAR, FAR)),
then return sum((sil - image_ref)^2).

Reformulation: each barycentric weight w_i of face f is an *affine* function
of the pixel NDC coords, w_i = a_i*x + b_i*y + c_i, so
    covered(p) = max_f min_i w_i(p, f) > 0.
The depth test is provably redundant when every camera-space vertex z lies
inside (NEAR, FAR); otherwise two extra affine maps are appended to the min.

Work pruning (host-side, exact):
  - A pixel strictly outside the global face bounding box can never be
    covered; its loss term ref^2 is summed on the host.
  - The in-bbox area is cut into 16x8-pixel blocks (= one 128-lane tile
    each). Each block only needs faces whose bbox overlaps it.
  - A block that lies fully inside a single valid face (all affine maps
    strictly positive at the block's 4 corner pixel centers, hence by
    linearity at every interior pixel center) is fully covered: the host
    adds sum((1-ref)^2) for it and the device never sees it.  This culls
    the (large) silhouette interior, leaving only boundary-ish blocks.
  - Remaining active blocks are sorted by face count and snake-dealt to
    the 8 cores, so all cores run an identical slot schedule (SPMD) whose
    per-slot face capacity is the max count in the 8-block group (pad 8).

Device (SPMD, one program on 8 cores; schedule baked at build time):
  - PE: per slot one K=9 bf16 matmul (lhsT = pixel matrix [9, 128],
    rhs = coefficients [9, 3*cap]) -> PSUM.  Small slots are packed S per
    PSUM bank (3*cap*S <= 504); large slots use a 3-bank tile with one
    matmul per affine map.  Each fp32 coefficient is split into 3 bf16
    components (exact to ~2^-25); pixel coords are exactly representable
    in bf16, so fp32 PSUM accumulation reproduces fp32 affine values
    essentially exactly.
  - ACT: stages maps 0,1 PSUM->SBUF as bf16 in one strided copy (the DVE
    reads at most one PSUM operand per instruction; sign-exact suffices).
  - DVE: min(w0,w1) at 2x (both SBUF bf16), min(.,w2) vs PSUM, grouped
    reduce_max over faces -> per-slot maxima; epilogue computes
    sum(((mx>0) - ref)^2) per partition row.
  - No end-of-kernel drain/barrier/sem-clear: the NEFF-end quiesce covers
    DMA completion, and the runtime reinitializes semaphores per load
    (verified empirically over repeated executions).
  - Host: sums 8x128 partials + the culled blocks' closed-form terms.
"""

import os
import sys
from contextlib import ExitStack

import numpy as np

for _p in (
    "/opt/trn_rl_repo",
    "/root/.axon_site",
    "/root/.axon_site/_ro/trn_rl_repo",
    "/root/.axon_site/_ro/pypackages",
):
    if os.path.isdir(_p) and _p not in sys.path:
        sys.path.append(_p)

import ml_dtypes  # noqa: E402

import concourse.bacc as bacc  # noqa: E402
import concourse.bass as bass  # noqa: E402
import concourse.tile as tile  # noqa: E402
from concourse import mybir  # noqa: E402
from concourse.alu_op_type import AluOpType  # noqa: E402
from concourse.bass_utils import run_bass_kernel_spmd  # noqa: E402

IS = 256
NEAR, FAR = 0.1, 100.0
VIEW_ANGLE_DEG = 30.0
CAM_DIST, ELEV, AZIM = 2.732, 0.0, 90.0
EPS = 1e-9

NCORES = 8
PTILE = 128                  # pixels per tile slot (partition dim)
BH, BW = 16, 8               # pixel block shape (BH*BW == PTILE)
PAD = 8                      # face-count padding granularity
MAXPK = 504                  # max packed-item columns (one PSUM bank, 3|MAXPK)
MAXCAP = 504                 # max faces per single matmul (<= 512 bank cols)
KSPLIT = 3                   # bf16 components per fp32 coefficient
K = 3 * KSPLIT               # matmul contraction dim
DUMMY_XY = -4.0              # off-screen coord for padding pixels

_prog_cache: dict = {}


class LeanTileContext(tile.TileContext):
    """TileContext without the end-of-kernel drain/barrier/sem-clear.

    The stock ending (drain + barriers + per-sem clears) costs ~10us.  The
    runtime's NEFF-end quiesce already waits for outstanding DMA, and
    semaphores are reinitialized on load, so correctness holds across
    repeated executions (verified on hardware).
    """

    def _drain_and_barrier(self, tick_clock, wait_clock):
        popped = self.nc._tile_sem_poison_stack.pop()
        assert popped is self._sem_poison


def _camera_transform(v: np.ndarray) -> np.ndarray:
    """Replicate reference's look_at + perspective in fp32. v: [V,3]."""
    e, a = np.radians(ELEV), np.radians(AZIM)
    eye = np.array(
        [
            CAM_DIST * np.cos(e) * np.sin(a),
            CAM_DIST * np.sin(e),
            -CAM_DIST * np.cos(e) * np.cos(a),
        ],
        dtype=np.float32,
    )
    at = np.zeros(3, np.float32)
    up = np.array([0.0, 1.0, 0.0], np.float32)
    z = at - eye
    z = (z / np.linalg.norm(z)).astype(np.float32)
    x = np.cross(up, z)
    x = (x / np.linalg.norm(x)).astype(np.float32)
    y = np.cross(z, x)
    y = (y / np.linalg.norm(y)).astype(np.float32)
    R = np.stack([x, y, z]).astype(np.float32)
    vc = ((v - eye) @ R.T).astype(np.float32)
    w = np.float32(np.tan(np.radians(VIEW_ANGLE_DEG)))
    zc = vc[:, 2]
    return np.stack([vc[:, 0] / (zc * w), vc[:, 1] / (zc * w), zc], -1).astype(
        np.float32
    )


def _face_coefficients(fv: np.ndarray):
    """Affine coefficients per map: returns (coeffs [nmaps,3,F] f32,
    valid [F] bool, nmaps)."""
    F = fv.shape[0]
    x0, x1, x2 = fv[:, 0, 0], fv[:, 1, 0], fv[:, 2, 0]
    y0, y1, y2 = fv[:, 0, 1], fv[:, 1, 1], fv[:, 2, 1]
    z0, z1, z2 = fv[:, 0, 2], fv[:, 1, 2], fv[:, 2, 2]

    denom = (y1 - y2) * (x0 - x2) + (x2 - x1) * (y0 - y2)
    valid = (np.abs(denom) > EPS) & np.all(np.isfinite(fv.reshape(F, -1)), -1)
    d = np.where(valid, denom, np.float32(1.0)).astype(np.float32)

    a0 = (y1 - y2) / d
    b0 = (x2 - x1) / d
    c0 = -(a0 * x2 + b0 * y2)
    a1 = (y2 - y0) / d
    b1 = (x0 - x2) / d
    c1 = -(a1 * x2 + b1 * y2)
    a2 = -(a0 + a1)
    b2 = -(b0 + b1)
    c2 = np.float32(1.0) - c0 - c1

    # Depth redundancy: for an interior pixel the perspective-correct depth
    # is a harmonic mean of vertex z's, hence inside (NEAR, FAR) whenever
    # all (valid-face) vertex z's are.
    z_valid = fv[valid][:, :, 2] if valid.any() else np.array([[1.0]])
    depth_safe = bool(
        np.all((z_valid > NEAR * 1.0001) & (z_valid < FAR * 0.9999)))

    maps = [(a0, b0, c0), (a1, b1, c1), (a2, b2, c2)]
    if not depth_safe:
        iz0 = np.float32(1.0) / z0
        iz1 = np.float32(1.0) / z1
        iz2 = np.float32(1.0) / z2
        az = a0 * iz0 + a1 * iz1 + a2 * iz2
        bz = b0 * iz0 + b1 * iz1 + b2 * iz2
        cz = c0 * iz0 + c1 * iz1 + c2 * iz2
        maps.append((az, bz, cz - np.float32(1.0 / FAR)))
        maps.append((-az, -bz, np.float32(1.0 / NEAR) - cz))

    nmaps = len(maps)
    coeffs = np.empty((nmaps, 3, F), np.float32)
    for m, (a, b, c) in enumerate(maps):
        bad = ~(valid & np.isfinite(a) & np.isfinite(b) & np.isfinite(c))
        coeffs[m, 0] = np.where(bad, np.float32(0.0), a)
        coeffs[m, 1] = np.where(bad, np.float32(0.0), b)
        coeffs[m, 2] = np.where(bad, np.float32(-1.0), c)
    return coeffs, valid, nmaps


def _split_bf16(v: np.ndarray) -> list[np.ndarray]:
    """Split fp32 array into KSPLIT bf16 components summing to ~v (2^-25)."""
    parts = []
    rem = v.astype(np.float32)
    for _ in range(KSPLIT):
        p = rem.astype(ml_dtypes.bfloat16)
        parts.append(p)
        rem = (rem - p.astype(np.float32)).astype(np.float32)
    return parts


def _make_items(nmaps: int, caps: tuple[int, ...]):
    """Group slots (face capacities, desc order) into device work items.

    ("p", cap, S, j0): S slots j0..j0+S-1, each padded to cap columns;
        one matmul per slot into a shared PSUM bank (nmaps*cap*S <= MAXPK).
    ("b", cap, j, chunks): slot j processed as len(chunks) chunks, each
        chunk a per-map matmul group into a 3-bank tile (cap <= MAXCAP
        per chunk); chunks[i] is the chunk's capacity.
    """
    items = []
    NT = len(caps)
    maxpk = (MAXPK // nmaps // PAD) * PAD * nmaps  # pack budget, PAD-aligned
    j = 0
    while j < NT:
        c = caps[j]
        if nmaps * c <= maxpk:
            S = 1
            while j + S < NT and nmaps * c * (S + 1) <= maxpk:
                S += 1
            items.append(("p", c, S, j))
            j += S
        else:
            nch = int(np.ceil(c / MAXCAP))
            ch = int(np.ceil(c / nch / PAD)) * PAD
            chunks = []
            left = c
            while left > 0:
                chunks.append(min(ch, max(PAD, left)))
                left -= chunks[-1]
            items.append(("b", c, j, tuple(chunks)))
            j += 1
    return items


def _make_schedule(vertices, image_ref, faces):
    """Host planning: prune + block + cull + deal.

    Returns (in_maps, nmaps, caps, items, host_extra)."""
    v = np.asarray(vertices, np.float32)[0]
    f = np.asarray(faces)[0].astype(np.int64)
    img = np.asarray(image_ref, np.float32)[0]
    img_flat = img.reshape(-1)

    vp = _camera_transform(v)
    fv = vp[f]                                    # [F,3,3]
    coeffs, valid, nmaps = _face_coefficients(fv)
    F = fv.shape[0]

    i = np.arange(IS, dtype=np.float32)
    xcol = (2.0 * i + 1.0 - IS) / IS
    yrow = (2.0 * (IS - 1.0 - i) + 1.0 - IS) / IS   # decreasing in row
    marg = np.float32(2.0 / IS)                     # one-pixel margin

    vi = np.where(valid)[0]
    if len(vi):
        fx = fv[:, :, 0]
        fy = fv[:, :, 1]
        fxmin, fxmax = fx.min(1), fx.max(1)
        fymin, fymax = fy.min(1), fy.max(1)
        gxmin, gxmax = fxmin[vi].min(), fxmax[vi].max()
        gymin, gymax = fymin[vi].min(), fymax[vi].max()
        rows = np.where((yrow >= gymin - marg) & (yrow <= gymax + marg))[0]
        cols = np.where((xcol >= gxmin - marg) & (xcol <= gxmax + marg))[0]
    else:
        rows = cols = np.array([], np.int64)

    A = coeffs[:, 0, :]                           # [nmaps, F]
    B = coeffs[:, 1, :]
    C = coeffs[:, 2, :]

    blocks = []        # active: (count, face_idx, pixel_idx)
    covered_extra = np.float32(0.0)
    handled = np.zeros(IS * IS, bool)  # covered-block pixels (host-summed)
    if len(rows) and len(cols):
        r0, r1 = int(rows.min()), int(rows.max()) + 1
        c0, c1 = int(cols.min()), int(cols.max()) + 1
        for rr in range(r0, r1, BH):
            for cc in range(c0, c1, BW):
                rr2, cc2 = min(rr + BH, r1), min(cc + BW, c1)
                ylo, yhi = yrow[rr2 - 1] - marg, yrow[rr] + marg
                xlo, xhi = xcol[cc] - marg, xcol[cc2 - 1] + marg
                inter = valid & (fymax >= ylo) & (fymin <= yhi) \
                    & (fxmax >= xlo) & (fxmin <= xhi)
                fl = np.where(inter)[0]
                if not len(fl):
                    continue          # pixels stay unassigned -> host ref^2
                rgrid, cgrid = np.meshgrid(np.arange(rr, rr2),
                                           np.arange(cc, cc2), indexing="ij")
                px = (rgrid * IS + cgrid).reshape(-1)
                # full-coverage cull: some face has every affine map
                # strictly positive at all 4 corner pixel centers
                cx = np.array([xcol[cc], xcol[cc2 - 1]], np.float32)
                cy = np.array([yrow[rr], yrow[rr2 - 1]], np.float32)
                CX, CY = np.meshgrid(cx, cy)
                P0, P1 = CX.ravel(), CY.ravel()   # [4]
                W = (A[:, fl, None] * P0[None, None, :]
                     + B[:, fl, None] * P1[None, None, :]
                     + C[:, fl, None])            # [nmaps, nf, 4]
                if bool(np.any((W > 1e-6).all(axis=(0, 2)))):
                    covered_extra += np.sum(
                        np.square(np.float32(1.0) - img_flat[px]),
                        dtype=np.float32)
                    handled[px] = True
                    continue
                blocks.append((len(fl), fl, px))

    if not blocks:
        blocks = [(0, np.array([], np.int64), np.array([], np.int64))]

    blocks.sort(key=lambda b: -b[0])
    NT = (len(blocks) + NCORES - 1) // NCORES
    empty = (0, np.array([], np.int64), np.array([], np.int64))
    while len(blocks) < NT * NCORES:
        blocks.append(empty)

    caps = []
    for j in range(NT):
        grp = blocks[NCORES * j:NCORES * (j + 1)]
        caps.append(max(PAD, int(np.ceil(max(b[0] for b in grp) / PAD)) * PAD))
    caps = tuple(caps)
    items = _make_items(nmaps, caps)

    # per-slot column count in the coef stream (incl. padding), layouts:
    #   "p": per slot s: [m0 x cap | m1 x cap | m2 x cap]   (slot-major)
    #   "b": per chunk:  [m0 x ch | m1 x ch | m2 x ch]
    CW = 0
    for it in items:
        if it[0] == "p":
            CW += nmaps * it[1] * it[2]
        else:
            CW += nmaps * sum(it[3])

    # coefficient splits with a trailing dummy column (index F)
    csp = np.empty((nmaps, 3, KSPLIT, F + 1), ml_dtypes.bfloat16)
    for m in range(nmaps):
        for j3 in range(3):
            col = np.concatenate(
                [coeffs[m, j3],
                 [np.float32(-1.0 if j3 == 2 else 0.0)]])
            for s, part in enumerate(_split_bf16(col)):
                csp[m, j3, s] = part

    PIXW = NT * PTILE
    assigned = handled
    in_maps = []
    for k in range(NCORES):
        pix = np.full((K, PIXW), np.float32(DUMMY_XY), np.float32)
        ref = np.zeros((PTILE, NT), np.float32)
        coef = np.empty((K, CW), ml_dtypes.bfloat16)
        # pixels + ref per slot
        slot_fidx = []
        for j in range(NT):
            cnt, fl, px = blocks[NCORES * j + k]
            npx = len(px)
            if npx:
                lane_x = xcol[px % IS]
                lane_y = yrow[px // IS]
                for s in range(KSPLIT):
                    pix[s * 3 + 0, j * PTILE:j * PTILE + npx] = lane_x
                    pix[s * 3 + 1, j * PTILE:j * PTILE + npx] = lane_y
                ref[:npx, j] = img_flat[px]
                assigned[px] = True
            for s in range(KSPLIT):
                pix[s * 3 + 2, j * PTILE:(j + 1) * PTILE] = 1.0
            slot_fidx.append((cnt, fl))
        # coefficients per item
        cb = 0
        for it in items:
            if it[0] == "p":
                _, cap, S, j0 = it
                for s in range(S):
                    cnt, fl = slot_fidx[j0 + s]
                    fidx = np.full(cap, F, np.int64)
                    fidx[:cnt] = fl
                    for m in range(nmaps):
                        for ks in range(KSPLIT):
                            for j3 in range(3):
                                coef[ks * 3 + j3,
                                     cb + m * cap:cb + (m + 1) * cap] = \
                                    csp[m, j3, ks][fidx]
                    cb += nmaps * cap
            else:
                _, cap, j, chunks = it
                cnt, fl = slot_fidx[j]
                fidx_all = np.full(sum(chunks), F, np.int64)
                fidx_all[:cnt] = fl
                pos = 0
                for ch in chunks:
                    sel = fidx_all[pos:pos + ch]
                    for m in range(nmaps):
                        for ks in range(KSPLIT):
                            for j3 in range(3):
                                coef[ks * 3 + j3,
                                     cb + m * ch:cb + (m + 1) * ch] = \
                                    csp[m, j3, ks][sel]
                    cb += nmaps * ch
                    pos += ch
        assert cb == CW
        in_maps.append({
            "coef": np.concatenate(
                [pix.astype(ml_dtypes.bfloat16), coef], axis=1),
            "ref": ref,
        })

    host_extra = float(np.sum(np.square(img_flat[~assigned]),
                              dtype=np.float32) + covered_extra)
    return in_maps, nmaps, caps, items, host_extra


def _build_program(nmaps: int, caps, items) -> bass.Bass:
    NT = len(caps)
    PIXW = NT * PTILE
    CW = 0
    for it in items:
        CW += nmaps * (it[1] * it[2] if it[0] == "p" else sum(it[3]))
    # accumulator columns: one per slot + one per extra chunk
    nextra = sum(len(it[3]) - 1 for it in items if it[0] == "b")
    NACC = NT + nextra

    nc = bacc.Bacc()
    coef_d = nc.dram_tensor("coef", [K, PIXW + CW], mybir.dt.bfloat16,
                            kind="ExternalInput")
    out_d = nc.dram_tensor("out", [PTILE, NACC], mybir.dt.float32,
                           kind="ExternalOutput")

    with LeanTileContext(nc) as tc:
        with ExitStack() as ctx:
            const = ctx.enter_context(tc.tile_pool(name="const", bufs=1))
            total = PIXW + CW
            # part0 (sync): pixels + first item's coef; part1 (scalar,
            # overlaps the ACT table load): the rest.
            it0 = items[0]
            first_cols = nmaps * (it0[1] * it0[2] if it0[0] == "p"
                                  else sum(it0[3]))
            cut = min(PIXW + first_cols, total)
            cf = const.tile([K, total], mybir.dt.bfloat16)
            nc.sync.dma_start(cf[:, :cut], coef_d[:, :cut])
            if cut < total:
                nc.scalar.dma_start(cf[:, cut:], coef_d[:, cut:])
            mx = const.tile([PTILE, NACC], mybir.dt.float32)

            psum = ctx.enter_context(
                tc.tile_pool(name="psum", bufs=2, space="PSUM"))
            tmp = ctx.enter_context(tc.tile_pool(name="tmp", bufs=3))

            eidx = NT
            for it in items:
                if it[0] == "p":
                    _, cap, S, j0 = it
                    N = nmaps * cap
                    wp = psum.tile([PTILE, 512], mybir.dt.float32,
                                   tag="pk", bufs=2)
                    for s in range(S):
                        j = j0 + s
                        off = PIXW + _coef_off(nmaps, items, it) \
                            + s * N
                        nc.tensor.matmul(
                            wp[:, s * N:(s + 1) * N],
                            cf[:K, j * PTILE:(j + 1) * PTILE],
                            cf[:K, off:off + N],
                            start=True, stop=True)
                    wv = wp[:, :S * N].rearrange(
                        "p (s m b) -> p s m b", s=S, m=nmaps)
                    st = tmp.tile([PTILE, 1024], mybir.dt.bfloat16, tag="st")
                    # dst layout (m, s, b): all w0 contiguous, then all w1
                    stv = st[:, :S * 2 * cap].rearrange(
                        "p (m s b) -> p s m b", m=2, s=S)
                    nc.scalar.copy(stv, wv[:, :, 0:2, :])
                    mn = tmp.tile([PTILE, 512], mybir.dt.bfloat16, tag="mn")
                    mnv = mn[:, :S * cap].rearrange(
                        "p (s b) -> p s b", s=S)
                    nc.vector.tensor_tensor(mn[:, :S * cap],
                                            st[:, :S * cap],
                                            st[:, S * cap:2 * S * cap],
                                            op=AluOpType.min)
                    for m in range(2, nmaps):
                        nc.vector.tensor_tensor(mnv, mnv, wv[:, :, m, :],
                                                op=AluOpType.min)
                    nc.vector.reduce_max(mx[:, j0:j0 + S], mnv,
                                         axis=mybir.AxisListType.X)
                else:
                    _, cap, j, chunks = it
                    off = PIXW + _coef_off(nmaps, items, it)
                    for ci, ch in enumerate(chunks):
                        wp = psum.tile([PTILE, 1536], mybir.dt.float32,
                                       tag="big", bufs=2)
                        for m in range(nmaps):
                            nc.tensor.matmul(
                                wp[:, 512 * m:512 * m + ch],
                                cf[:K, j * PTILE:(j + 1) * PTILE],
                                cf[:K, off + m * ch:off + (m + 1) * ch],
                                start=True, stop=True)
                        off += nmaps * ch
                        st = tmp.tile([PTILE, 1024], mybir.dt.bfloat16,
                                      tag="st")
                        stv = st[:, :2 * ch].rearrange(
                            "p (m b) -> p m b", m=2)
                        nc.scalar.copy(
                            stv,
                            wp[:].rearrange(
                                "p (m b) -> p m b", m=3)[:, 0:2, :ch])
                        mn = tmp.tile([PTILE, 512], mybir.dt.bfloat16,
                                      tag="mn")
                        nc.vector.tensor_tensor(mn[:, :ch], stv[:, 0, :],
                                                stv[:, 1, :],
                                                op=AluOpType.min)
                        for m in range(2, nmaps):
                            nc.vector.tensor_tensor(
                                mn[:, :ch], mn[:, :ch],
                                wp[:, 512 * m:512 * m + ch],
                                op=AluOpType.min)
                        dst = mx[:, j:j + 1] if ci == 0 \
                            else mx[:, eidx:eidx + 1]
                        nc.vector.reduce_max(dst, mn[:, :ch],
                                             axis=mybir.AxisListType.X)
                        if ci > 0:
                            eidx += 1

            if NACC > NT + nextra:
                # keep the pad column initialized so the out DMA is defined
                nc.vector.tensor_copy(mx[:, NT + nextra:], mx[:, :1])
            nc.sync.dma_start(out_d[:], mx[:])
    nc.compile()
    return nc


def _coef_off(nmaps: int, items, target) -> int:
    off = 0
    for it in items:
        if it is target:
            return off
        off += nmaps * (it[1] * it[2] if it[0] == "p" else sum(it[3]))
    raise KeyError(target)


def run_sharded(vertices, image_ref, faces, trace=False, **spmd_kwargs):
    """Runs the SPMD kernel on 8 cores; returns (loss, BassKernelResults)."""
    in_maps, nmaps, caps, items, host_extra = _make_schedule(
        vertices, image_ref, faces)
    key = (nmaps, caps, tuple(items))
    if key not in _prog_cache:
        _prog_cache[key] = _build_program(nmaps, caps, items)
    nc = _prog_cache[key]
    dev_maps = [{"coef": m["coef"]} for m in in_maps]
    results = run_bass_kernel_spmd(
        nc, dev_maps, core_ids=list(range(NCORES)), trace=trace,
        **spmd_kwargs)
    loss = _host_loss(in_maps, [r["out"] for r in results.results],
                      caps, items, host_extra)
    return loss, results


def _host_loss(in_maps, outs, caps, items, host_extra) -> np.float32:
    NT = len(caps)
    # slot -> accumulator column list (chunked slots own extra columns)
    slot_cols = [[j] for j in range(NT)]
    eidx = NT
    for it in items:
        if it[0] == "b":
            for _ in range(len(it[3]) - 1):
                slot_cols[it[2]].append(eidx)
                eidx += 1
    loss = np.float32(host_extra)
    for m, out in zip(in_maps, outs):
        mx = np.asarray(out, np.float32)               # [128, NACC]
        cov = np.zeros((PTILE, NT), np.float32)
        for j in range(NT):
            cov[:, j] = (mx[:, slot_cols[j]] > 0.0).any(axis=1)
        diff = cov - m["ref"]
        loss = np.float32(loss + np.sum(diff * diff, dtype=np.float32))
    return loss


def kernel(vertices: np.ndarray, image_ref: np.ndarray,
           faces: np.ndarray) -> np.ndarray:
    loss, _ = run_sharded(vertices, image_ref, faces, trace=False)
    return np.asarray(loss, dtype=np.float32)


# revision 10
# speedup vs baseline: 2.3329x; 1.0904x over previous
"""Trainium2 Bass kernel for the neural-renderer silhouette MSE loss.

Reference computation: project 512 vertices, gather 1024 triangle faces,
rasterize a 256x256 silhouette (a pixel is covered iff it lies strictly
inside some valid face and the perspective-correct depth is in (NEAR, FAR)),
then return sum((sil - image_ref)^2).

Reformulation: each barycentric weight w_i of face f is an *affine* function
of the pixel NDC coords, w_i = a_i*x + b_i*y + c_i, so
    covered(p) = max_f min_i w_i(p, f) > 0.
The depth test is provably redundant when every camera-space vertex z lies
inside (NEAR, FAR); otherwise two extra affine maps are appended to the min.

Work pruning (host-side, exact):
  - A pixel strictly outside the global face bounding box can never be
    covered; its loss term ref^2 is summed on the host.
  - The in-bbox area is cut into 16x8-pixel blocks (= one 128-lane tile
    each). Each block only needs faces whose bbox overlaps it.
  - A block that lies fully inside a single valid face (all affine maps
    strictly positive at the block's 4 corner pixel centers, hence by
    linearity at every interior pixel center) is fully covered: the host
    adds sum((1-ref)^2) for it and the device never sees it.  This culls
    the (large) silhouette interior, leaving only boundary-ish blocks.
  - Remaining active blocks are sorted by face count and snake-dealt to
    the 8 cores, so all cores run an identical slot schedule (SPMD) whose
    per-slot face capacity is the max count in the 8-block group (pad 8).

Device (SPMD, one program on 8 cores; schedule baked at build time):
  - PE: per slot one K=9 bf16 matmul (lhsT = pixel matrix [9, 128],
    rhs = coefficients [9, 3*cap]) -> PSUM.  Small slots are packed S per
    PSUM bank (3*cap*S <= 504); large slots use a 3-bank tile with one
    matmul per affine map.  Each fp32 coefficient is split into 3 bf16
    components (exact to ~2^-25); pixel coords are exactly representable
    in bf16, so fp32 PSUM accumulation reproduces fp32 affine values
    essentially exactly.
  - ACT: stages maps 0,1 PSUM->SBUF as bf16 in one strided copy (the DVE
    reads at most one PSUM operand per instruction; sign-exact suffices).
  - DVE: min(w0,w1) at 2x (both SBUF bf16), min(.,w2) vs PSUM, grouped
    reduce_max over faces -> per-slot maxima; epilogue computes
    sum(((mx>0) - ref)^2) per partition row.
  - No end-of-kernel drain/barrier/sem-clear: the NEFF-end quiesce covers
    DMA completion, and the runtime reinitializes semaphores per load
    (verified empirically over repeated executions).
  - Host: sums 8x128 partials + the culled blocks' closed-form terms.
"""

import os
import sys
from contextlib import ExitStack

import numpy as np

for _p in (
    "/opt/trn_rl_repo",
    "/root/.axon_site",
    "/root/.axon_site/_ro/trn_rl_repo",
    "/root/.axon_site/_ro/pypackages",
):
    if os.path.isdir(_p) and _p not in sys.path:
        sys.path.append(_p)

import ml_dtypes  # noqa: E402

import concourse.bacc as bacc  # noqa: E402
import concourse.bass as bass  # noqa: E402
import concourse.tile as tile  # noqa: E402
from concourse import mybir  # noqa: E402
from concourse.alu_op_type import AluOpType  # noqa: E402
from concourse.bass_utils import run_bass_kernel_spmd  # noqa: E402

IS = 256
NEAR, FAR = 0.1, 100.0
VIEW_ANGLE_DEG = 30.0
CAM_DIST, ELEV, AZIM = 2.732, 0.0, 90.0
EPS = 1e-9

NCORES = 8
PTILE = 128                  # pixels per tile slot (partition dim)
BH, BW = 16, 8               # pixel block shape (BH*BW == PTILE)
PAD = 8                      # face-count padding granularity
MAXPK = 504                  # max packed-item columns (one PSUM bank, 3|MAXPK)
MAXCAP = 504                 # max faces per single matmul (<= 512 bank cols)
KSPLIT = 3                   # bf16 components per fp32 coefficient
K = 3 * KSPLIT               # matmul contraction dim
DUMMY_XY = -4.0              # off-screen coord for padding pixels

_prog_cache: dict = {}


class LeanTileContext(tile.TileContext):
    """TileContext without the end-of-kernel drain/barrier/sem-clear.

    The stock ending (drain + barriers + per-sem clears) costs ~10us.  The
    runtime's NEFF-end quiesce already waits for outstanding DMA, and
    semaphores are reinitialized on load, so correctness holds across
    repeated executions (verified on hardware).
    """

    def _drain_and_barrier(self, tick_clock, wait_clock):
        popped = self.nc._tile_sem_poison_stack.pop()
        assert popped is self._sem_poison


def _camera_transform(v: np.ndarray) -> np.ndarray:
    """Replicate reference's look_at + perspective in fp32. v: [V,3]."""
    e, a = np.radians(ELEV), np.radians(AZIM)
    eye = np.array(
        [
            CAM_DIST * np.cos(e) * np.sin(a),
            CAM_DIST * np.sin(e),
            -CAM_DIST * np.cos(e) * np.cos(a),
        ],
        dtype=np.float32,
    )
    at = np.zeros(3, np.float32)
    up = np.array([0.0, 1.0, 0.0], np.float32)
    z = at - eye
    z = (z / np.linalg.norm(z)).astype(np.float32)
    x = np.cross(up, z)
    x = (x / np.linalg.norm(x)).astype(np.float32)
    y = np.cross(z, x)
    y = (y / np.linalg.norm(y)).astype(np.float32)
    R = np.stack([x, y, z]).astype(np.float32)
    vc = ((v - eye) @ R.T).astype(np.float32)
    w = np.float32(np.tan(np.radians(VIEW_ANGLE_DEG)))
    zc = vc[:, 2]
    return np.stack([vc[:, 0] / (zc * w), vc[:, 1] / (zc * w), zc], -1).astype(
        np.float32
    )


def _face_coefficients(fv: np.ndarray):
    """Affine coefficients per map: returns (coeffs [nmaps,3,F] f32,
    valid [F] bool, nmaps)."""
    F = fv.shape[0]
    x0, x1, x2 = fv[:, 0, 0], fv[:, 1, 0], fv[:, 2, 0]
    y0, y1, y2 = fv[:, 0, 1], fv[:, 1, 1], fv[:, 2, 1]
    z0, z1, z2 = fv[:, 0, 2], fv[:, 1, 2], fv[:, 2, 2]

    denom = (y1 - y2) * (x0 - x2) + (x2 - x1) * (y0 - y2)
    valid = (np.abs(denom) > EPS) & np.all(np.isfinite(fv.reshape(F, -1)), -1)
    d = np.where(valid, denom, np.float32(1.0)).astype(np.float32)

    a0 = (y1 - y2) / d
    b0 = (x2 - x1) / d
    c0 = -(a0 * x2 + b0 * y2)
    a1 = (y2 - y0) / d
    b1 = (x0 - x2) / d
    c1 = -(a1 * x2 + b1 * y2)
    a2 = -(a0 + a1)
    b2 = -(b0 + b1)
    c2 = np.float32(1.0) - c0 - c1

    # Depth redundancy: for an interior pixel the perspective-correct depth
    # is a harmonic mean of vertex z's, hence inside (NEAR, FAR) whenever
    # all (valid-face) vertex z's are.
    z_valid = fv[valid][:, :, 2] if valid.any() else np.array([[1.0]])
    depth_safe = bool(
        np.all((z_valid > NEAR * 1.0001) & (z_valid < FAR * 0.9999)))

    maps = [(a0, b0, c0), (a1, b1, c1), (a2, b2, c2)]
    if not depth_safe:
        iz0 = np.float32(1.0) / z0
        iz1 = np.float32(1.0) / z1
        iz2 = np.float32(1.0) / z2
        az = a0 * iz0 + a1 * iz1 + a2 * iz2
        bz = b0 * iz0 + b1 * iz1 + b2 * iz2
        cz = c0 * iz0 + c1 * iz1 + c2 * iz2
        maps.append((az, bz, cz - np.float32(1.0 / FAR)))
        maps.append((-az, -bz, np.float32(1.0 / NEAR) - cz))

    nmaps = len(maps)
    coeffs = np.empty((nmaps, 3, F), np.float32)
    for m, (a, b, c) in enumerate(maps):
        bad = ~(valid & np.isfinite(a) & np.isfinite(b) & np.isfinite(c))
        coeffs[m, 0] = np.where(bad, np.float32(0.0), a)
        coeffs[m, 1] = np.where(bad, np.float32(0.0), b)
        coeffs[m, 2] = np.where(bad, np.float32(-1.0), c)
    return coeffs, valid, nmaps


def _split_bf16(v: np.ndarray) -> list[np.ndarray]:
    """Split fp32 array into KSPLIT bf16 components summing to ~v (2^-25)."""
    parts = []
    rem = v.astype(np.float32)
    for _ in range(KSPLIT):
        p = rem.astype(ml_dtypes.bfloat16)
        parts.append(p)
        rem = (rem - p.astype(np.float32)).astype(np.float32)
    return parts


def _make_items(nmaps: int, caps: tuple[int, ...]):
    """Group slots (face capacities, desc order) into device work items.

    ("p", cap, S, j0): S slots j0..j0+S-1, each padded to cap columns;
        one matmul per slot into a shared PSUM bank (nmaps*cap*S <= MAXPK).
    ("b", cap, j, chunks): slot j processed as len(chunks) chunks, each
        chunk a per-map matmul group into a 3-bank tile (cap <= MAXCAP
        per chunk); chunks[i] is the chunk's capacity.
    """
    items = []
    NT = len(caps)
    maxpk = (MAXPK // nmaps // PAD) * PAD * nmaps  # pack budget, PAD-aligned
    j = 0
    while j < NT:
        c = caps[j]
        if nmaps * c <= maxpk:
            S = 1
            while j + S < NT and nmaps * c * (S + 1) <= maxpk:
                S += 1
            items.append(("p", c, S, j))
            j += S
        else:
            nch = int(np.ceil(c / MAXCAP))
            ch = int(np.ceil(c / nch / PAD)) * PAD
            chunks = []
            left = c
            while left > 0:
                chunks.append(min(ch, max(PAD, left)))
                left -= chunks[-1]
            items.append(("b", c, j, tuple(chunks)))
            j += 1
    if len(items) > 1 and items[-1][0] == "p":
        items.insert(0, items.pop())
    return items


def _make_schedule(vertices, image_ref, faces):
    """Host planning: prune + block + cull + deal.

    Returns (in_maps, nmaps, caps, items, host_extra)."""
    v = np.asarray(vertices, np.float32)[0]
    f = np.asarray(faces)[0].astype(np.int64)
    img = np.asarray(image_ref, np.float32)[0]
    img_flat = img.reshape(-1)

    vp = _camera_transform(v)
    fv = vp[f]                                    # [F,3,3]
    coeffs, valid, nmaps = _face_coefficients(fv)
    F = fv.shape[0]

    i = np.arange(IS, dtype=np.float32)
    xcol = (2.0 * i + 1.0 - IS) / IS
    yrow = (2.0 * (IS - 1.0 - i) + 1.0 - IS) / IS   # decreasing in row
    marg = np.float32(2.0 / IS)                     # one-pixel margin

    vi = np.where(valid)[0]
    if len(vi):
        fx = fv[:, :, 0]
        fy = fv[:, :, 1]
        fxmin, fxmax = fx.min(1), fx.max(1)
        fymin, fymax = fy.min(1), fy.max(1)
        gxmin, gxmax = fxmin[vi].min(), fxmax[vi].max()
        gymin, gymax = fymin[vi].min(), fymax[vi].max()
        rows = np.where((yrow >= gymin - marg) & (yrow <= gymax + marg))[0]
        cols = np.where((xcol >= gxmin - marg) & (xcol <= gxmax + marg))[0]
    else:
        rows = cols = np.array([], np.int64)

    A = coeffs[:, 0, :]                           # [nmaps, F]
    B = coeffs[:, 1, :]
    C = coeffs[:, 2, :]

    blocks = []        # active: (count, face_idx, pixel_idx)
    covered_extra = np.float32(0.0)
    handled = np.zeros(IS * IS, bool)  # covered-block pixels (host-summed)
    if len(rows) and len(cols):
        r0, r1 = int(rows.min()), int(rows.max()) + 1
        c0, c1 = int(cols.min()), int(cols.max()) + 1
        for rr in range(r0, r1, BH):
            for cc in range(c0, c1, BW):
                rr2, cc2 = min(rr + BH, r1), min(cc + BW, c1)
                ylo, yhi = yrow[rr2 - 1] - marg, yrow[rr] + marg
                xlo, xhi = xcol[cc] - marg, xcol[cc2 - 1] + marg
                inter = valid & (fymax >= ylo) & (fymin <= yhi) \
                    & (fxmax >= xlo) & (fxmin <= xhi)
                fl = np.where(inter)[0]
                if not len(fl):
                    continue          # pixels stay unassigned -> host ref^2
                rgrid, cgrid = np.meshgrid(np.arange(rr, rr2),
                                           np.arange(cc, cc2), indexing="ij")
                px = (rgrid * IS + cgrid).reshape(-1)
                # full-coverage cull: some face has every affine map
                # strictly positive at all 4 corner pixel centers
                cx = np.array([xcol[cc], xcol[cc2 - 1]], np.float32)
                cy = np.array([yrow[rr], yrow[rr2 - 1]], np.float32)
                CX, CY = np.meshgrid(cx, cy)
                P0, P1 = CX.ravel(), CY.ravel()   # [4]
                W = (A[:, fl, None] * P0[None, None, :]
                     + B[:, fl, None] * P1[None, None, :]
                     + C[:, fl, None])            # [nmaps, nf, 4]
                if bool(np.any((W > 1e-6).all(axis=(0, 2)))):
                    covered_extra += np.sum(
                        np.square(np.float32(1.0) - img_flat[px]),
                        dtype=np.float32)
                    handled[px] = True
                    continue
                blocks.append((len(fl), fl, px))

    if not blocks:
        blocks = [(0, np.array([], np.int64), np.array([], np.int64))]

    blocks.sort(key=lambda b: -b[0])
    NT = (len(blocks) + NCORES - 1) // NCORES
    empty = (0, np.array([], np.int64), np.array([], np.int64))
    while len(blocks) < NT * NCORES:
        blocks.append(empty)

    caps = []
    for j in range(NT):
        grp = blocks[NCORES * j:NCORES * (j + 1)]
        caps.append(max(PAD, int(np.ceil(max(b[0] for b in grp) / PAD)) * PAD))
    caps = tuple(caps)
    items = _make_items(nmaps, caps)

    # per-slot column count in the coef stream (incl. padding), layouts:
    #   "p": per slot s: [m0 x cap | m1 x cap | m2 x cap]   (slot-major)
    #   "b": per chunk:  [m0 x ch | m1 x ch | m2 x ch]
    CW = 0
    for it in items:
        if it[0] == "p":
            CW += nmaps * it[1] * it[2]
        else:
            CW += nmaps * sum(it[3])

    # coefficient splits with a trailing dummy column (index F)
    csp = np.empty((nmaps, 3, KSPLIT, F + 1), ml_dtypes.bfloat16)
    for m in range(nmaps):
        for j3 in range(3):
            col = np.concatenate(
                [coeffs[m, j3],
                 [np.float32(-1.0 if j3 == 2 else 0.0)]])
            for s, part in enumerate(_split_bf16(col)):
                csp[m, j3, s] = part

    PIXW = NT * PTILE
    assigned = handled
    in_maps = []
    for k in range(NCORES):
        pix = np.full((K, PIXW), np.float32(DUMMY_XY), np.float32)
        ref = np.zeros((PTILE, NT), np.float32)
        coef = np.empty((K, CW), ml_dtypes.bfloat16)
        # pixels + ref per slot
        slot_fidx = []
        for j in range(NT):
            cnt, fl, px = blocks[NCORES * j + k]
            npx = len(px)
            if npx:
                lane_x = xcol[px % IS]
                lane_y = yrow[px // IS]
                for s in range(KSPLIT):
                    pix[s * 3 + 0, j * PTILE:j * PTILE + npx] = lane_x
                    pix[s * 3 + 1, j * PTILE:j * PTILE + npx] = lane_y
                ref[:npx, j] = img_flat[px]
                assigned[px] = True
            for s in range(KSPLIT):
                pix[s * 3 + 2, j * PTILE:(j + 1) * PTILE] = 1.0
            slot_fidx.append((cnt, fl))
        # coefficients per item
        cb = 0
        for it in items:
            if it[0] == "p":
                _, cap, S, j0 = it
                for s in range(S):
                    cnt, fl = slot_fidx[j0 + s]
                    fidx = np.full(cap, F, np.int64)
                    fidx[:cnt] = fl
                    for m in range(nmaps):
                        for ks in range(KSPLIT):
                            for j3 in range(3):
                                coef[ks * 3 + j3,
                                     cb + m * cap:cb + (m + 1) * cap] = \
                                    csp[m, j3, ks][fidx]
                    cb += nmaps * cap
            else:
                _, cap, j, chunks = it
                cnt, fl = slot_fidx[j]
                fidx_all = np.full(sum(chunks), F, np.int64)
                fidx_all[:cnt] = fl
                pos = 0
                for ch in chunks:
                    sel = fidx_all[pos:pos + ch]
                    for m in range(nmaps):
                        for ks in range(KSPLIT):
                            for j3 in range(3):
                                coef[ks * 3 + j3,
                                     cb + m * ch:cb + (m + 1) * ch] = \
                                    csp[m, j3, ks][sel]
                    cb += nmaps * ch
                    pos += ch
        assert cb == CW
        in_maps.append({
            "coef": np.concatenate(
                [pix.astype(ml_dtypes.bfloat16), coef], axis=1),
            "ref": ref,
        })

    host_extra = float(np.sum(np.square(img_flat[~assigned]),
                              dtype=np.float32) + covered_extra)
    return in_maps, nmaps, caps, items, host_extra


def _build_program(nmaps: int, caps, items) -> bass.Bass:
    NT = len(caps)
    PIXW = NT * PTILE
    CW = 0
    for it in items:
        CW += nmaps * (it[1] * it[2] if it[0] == "p" else sum(it[3]))
    # accumulator columns: one per slot + one per extra chunk
    nextra = sum(len(it[3]) - 1 for it in items if it[0] == "b")
    NACC = NT + nextra

    nc = bacc.Bacc()
    coef_d = nc.dram_tensor("coef", [K, PIXW + CW], mybir.dt.bfloat16,
                            kind="ExternalInput")
    out_d = nc.dram_tensor("out", [PTILE, NACC], mybir.dt.float32,
                           kind="ExternalOutput")

    with LeanTileContext(nc) as tc:
        with ExitStack() as ctx:
            const = ctx.enter_context(tc.tile_pool(name="const", bufs=1))
            total = PIXW + CW
            # part0 (sync): pixels + first item's coef; part1 (scalar,
            # overlaps the ACT table load): the rest.
            it0 = items[0]
            first_cols = nmaps * (it0[1] * it0[2] if it0[0] == "p"
                                  else sum(it0[3]))
            cut = min(PIXW + first_cols, total)
            cf = const.tile([K, total], mybir.dt.bfloat16)
            nc.sync.dma_start(cf[:, :cut], coef_d[:, :cut])
            if cut < total:
                nc.scalar.dma_start(cf[:, cut:], coef_d[:, cut:])
            mx = const.tile([PTILE, NACC], mybir.dt.float32)

            psum = ctx.enter_context(
                tc.tile_pool(name="psum", bufs=2, space="PSUM"))
            tmp = ctx.enter_context(tc.tile_pool(name="tmp", bufs=3))

            eidx = NT
            for it in items:
                if it[0] == "p":
                    _, cap, S, j0 = it
                    N = nmaps * cap
                    wp = psum.tile([PTILE, 512], mybir.dt.float32,
                                   tag="pk", bufs=2)
                    for s in range(S):
                        j = j0 + s
                        off = PIXW + _coef_off(nmaps, items, it) \
                            + s * N
                        nc.tensor.matmul(
                            wp[:, s * N:(s + 1) * N],
                            cf[:K, j * PTILE:(j + 1) * PTILE],
                            cf[:K, off:off + N],
                            start=True, stop=True)
                    wv = wp[:, :S * N].rearrange(
                        "p (s m b) -> p s m b", s=S, m=nmaps)
                    st = tmp.tile([PTILE, 1536], mybir.dt.bfloat16, tag="st")
                    # dst layout (m, s, b): all w0, then all w1, then all w2
                    stv = st[:, :S * nmaps * cap].rearrange(
                        "p (m s b) -> p s m b", m=nmaps, s=S)
                    nc.scalar.copy(stv, wv)
                    mn = tmp.tile([PTILE, 512], mybir.dt.bfloat16, tag="mn")
                    mnv = mn[:, :S * cap].rearrange(
                        "p (s b) -> p s b", s=S)
                    Sc = S * cap
                    nc.vector.tensor_tensor(mn[:, :Sc], st[:, :Sc],
                                            st[:, Sc:2 * Sc],
                                            op=AluOpType.min)
                    for m in range(2, nmaps):
                        nc.vector.tensor_tensor(
                            mn[:, :Sc], mn[:, :Sc],
                            st[:, m * Sc:(m + 1) * Sc],
                            op=AluOpType.min)
                    nc.vector.reduce_max(mx[:, j0:j0 + S], mnv,
                                         axis=mybir.AxisListType.X)
                else:
                    _, cap, j, chunks = it
                    off = PIXW + _coef_off(nmaps, items, it)
                    for ci, ch in enumerate(chunks):
                        wp = psum.tile([PTILE, 1536], mybir.dt.float32,
                                       tag="big", bufs=2)
                        for m in range(nmaps):
                            nc.tensor.matmul(
                                wp[:, 512 * m:512 * m + ch],
                                cf[:K, j * PTILE:(j + 1) * PTILE],
                                cf[:K, off + m * ch:off + (m + 1) * ch],
                                start=True, stop=True)
                        off += nmaps * ch
                        st = tmp.tile([PTILE, 1536], mybir.dt.bfloat16,
                                      tag="st")
                        stv = st[:, :nmaps * ch].rearrange(
                            "p (m b) -> p m b", m=nmaps)
                        nc.scalar.copy(
                            stv,
                            wp[:].rearrange(
                                "p (m b) -> p m b", m=3)[:, :nmaps, :ch])
                        mn = tmp.tile([PTILE, 512], mybir.dt.bfloat16,
                                      tag="mn")
                        nc.vector.tensor_tensor(mn[:, :ch], st[:, :ch],
                                                st[:, ch:2 * ch],
                                                op=AluOpType.min)
                        for m in range(2, nmaps):
                            nc.vector.tensor_tensor(
                                mn[:, :ch], mn[:, :ch],
                                st[:, m * ch:(m + 1) * ch],
                                op=AluOpType.min)
                        dst = mx[:, j:j + 1] if ci == 0 \
                            else mx[:, eidx:eidx + 1]
                        nc.vector.reduce_max(dst, mn[:, :ch],
                                             axis=mybir.AxisListType.X)
                        if ci > 0:
                            eidx += 1

            nc.sync.dma_start(out_d[:], mx[:])
    nc.compile()
    return nc


def _coef_off(nmaps: int, items, target) -> int:
    off = 0
    for it in items:
        if it is target:
            return off
        off += nmaps * (it[1] * it[2] if it[0] == "p" else sum(it[3]))
    raise KeyError(target)


def run_sharded(vertices, image_ref, faces, trace=False, **spmd_kwargs):
    """Runs the SPMD kernel on 8 cores; returns (loss, BassKernelResults)."""
    in_maps, nmaps, caps, items, host_extra = _make_schedule(
        vertices, image_ref, faces)
    key = (nmaps, caps, tuple(items))
    if key not in _prog_cache:
        _prog_cache[key] = _build_program(nmaps, caps, items)
    nc = _prog_cache[key]
    dev_maps = [{"coef": m["coef"]} for m in in_maps]
    results = run_bass_kernel_spmd(
        nc, dev_maps, core_ids=list(range(NCORES)), trace=trace,
        **spmd_kwargs)
    loss = _host_loss(in_maps, [r["out"] for r in results.results],
                      caps, items, host_extra)
    return loss, results


def _host_loss(in_maps, outs, caps, items, host_extra) -> np.float32:
    NT = len(caps)
    # slot -> accumulator column list (chunked slots own extra columns)
    slot_cols = [[j] for j in range(NT)]
    eidx = NT
    for it in items:
        if it[0] == "b":
            for _ in range(len(it[3]) - 1):
                slot_cols[it[2]].append(eidx)
                eidx += 1
    loss = np.float32(host_extra)
    for m, out in zip(in_maps, outs):
        mx = np.asarray(out, np.float32)               # [128, NACC]
        cov = np.zeros((PTILE, NT), np.float32)
        for j in range(NT):
            cov[:, j] = (mx[:, slot_cols[j]] > 0.0).any(axis=1)
        diff = cov - m["ref"]
        loss = np.float32(loss + np.sum(diff * diff, dtype=np.float32))
    return loss


def kernel(vertices: np.ndarray, image_ref: np.ndarray,
           faces: np.ndarray) -> np.ndarray:
    loss, _ = run_sharded(vertices, image_ref, faces, trace=False)
    return np.asarray(loss, dtype=np.float32)
